# revision 11
# baseline (speedup 1.0000x reference)
"""Trainium2 Bass kernel for a 2-layer GCN encoder + edge dot-product decoder.

Math (matches the PyG-style reference):
    deg  = in-degree(dst)+1 (self loops), dinv = rsqrt(deg)
    A~[d,s] = dinv[s]*dinv[d] over edges+self-loops
    H1 = (A~ @ X) @ W1 + b1          (aggregate-first ordering)
    Z  = (A~ @ relu(H1) @ W2) + b2
    logits[e] = <Z[src_e], Z[dst_e]>

Distribution over 8 NeuronCores: nodes sharded via LPT balancing (staged
order), edges partitioned by destination owner, weights replicated.

Key measured constraint: every dynamic row-gather on the SWDGE path costs
~8.9ns/row regardless of batch/row size, serialized on GpSimd.  Design:
  * Layer-1 gather of x[src] rows is STATIC (indices known on host), so the
    host pre-gathers the edge stream into xgath (SBUF layout) and the device
    streams it with static DMA -- zero SWDGE.
  * Layer-2 must gather h2[src] (dynamic, 256B rows) -- batched dma_gather,
    <=1024 rows/call, lo/hi split for signed int16 indices.
  * The decoder runs in the SAME aggregation block structure: z[src] reuses
    the same index table (gathered from zfull), z[dst] is selected from the
    locally-kept z tile via host-shipped one-hot S01T matmuls (zero SWDGE),
    and the dot-products are batched mul + strided 3-D reduce.
The segment-sum scatter is a [128e x 128slot] matrix S (iota is_equal slot
* norm) applied on the Tensor engine with PSUM accumulation.
"""

import os

if os.environ.get("JAX_PLATFORMS") == "cpu":
    os.environ.pop("JAX_PLATFORMS")

import numpy as np

from concourse import bass, bacc, mybir, bass_utils
import concourse.tile as tile

# ---------------------------------------------------------------- sizes
N_NODES = 50000
N_EDGES = 400000
D_IN, D_H, D_OUT = 600, 628, 64
D_PAD = 640                     # x rows padded to 640 f16 = 1280 B
Z_PAD = 128                     # h2/z rows padded to 128 f16 = 256 B
C = 8
P = 128
HALF = 32768                    # int16 signed gather-index window size
B2V = 50176 - HALF              # second window base: [B2V, NALL), idx-B2V
MAXB = 8                        # blocks per dma_gather call (8*128=1024 idx)
GT = 4                          # tiles per GEMM group

F16 = mybir.dt.float16
F32 = mybir.dt.float32
I16 = mybir.dt.int16

NPC = N_NODES // C              # 6250
TILES = -(-NPC // P)            # 49
NPAD = TILES * P                # 6272
NALL = C * NPAD                 # 50176


def _chunks(total, step=128):
    out, o = [], 0
    while o < total:
        w = min(step, total - o)
        out.append((o, w))
        o += w
    return out


KCH = _chunks(D_IN)
MCH = _chunks(D_H)


def _wrap16(idx):
    """1-D int array (len % 128 == 0) -> [128, len/16] int16 wrapped layout:
    position i lives at (partition i%16, col i//16), tiled to 128 rows."""
    n = len(idx)
    t = np.asarray(idx, np.int64).reshape(n // 16, 16).T
    t = t.astype(np.uint16).view(np.int16)
    return np.tile(t, (8, 1))


# ---------------------------------------------------------------- host preprocessing
def _assign_nodes(d_all, N, tiles):
    """LPT-assign nodes to C*tiles buckets of <=128 slots, minimizing the
    max per-bucket edge count."""
    import heapq
    w = np.bincount(d_all, minlength=N)
    nb = C * tiles
    heap = [(0, b) for b in range(nb)]
    heapq.heapify(heap)
    cnt = np.zeros(nb, np.int64)
    nodec = np.empty(N, np.int64)
    nodet = np.empty(N, np.int64)
    nodesl = np.empty(N, np.int64)
    for n in np.argsort(-w, kind="stable"):
        while True:
            wt, b = heapq.heappop(heap)
            if cnt[b] < P:
                break
        nodec[n] = b // tiles
        nodet[n] = b % tiles
        nodesl[n] = cnt[b]
        cnt[b] += 1
        if cnt[b] < P:
            heapq.heappush(heap, (wt + int(w[n]), b))
    return nodec, nodet, nodesl


def _preprocess(x, edge_index, W1, b1, W2, b2):
    src = edge_index[0].astype(np.int64)
    dst = edge_index[1].astype(np.int64)
    loop = np.arange(N_NODES, dtype=np.int64)
    s_all = np.concatenate([src, loop])
    d_all = np.concatenate([dst, loop])
    deg = np.bincount(d_all, minlength=N_NODES).astype(np.float64)
    dinv = 1.0 / np.sqrt(deg)
    norm = (dinv[s_all] * dinv[d_all]).astype(np.float32)

    nodec, nodet, nodesl = _assign_nodes(dst, N_NODES, TILES)
    staged = nodec * NPAD + nodet * P + nodesl

    xs = np.zeros((NALL, D_PAD), dtype=np.float16)
    xs[staged, :D_IN] = x.astype(np.float16)

    # per-core staged x rows + self-loop weights (self-loops are folded into
    # the aggregation output instead of occupying block slots)
    inv_staged = np.zeros(NALL, np.int64)
    used = np.zeros(NALL, bool)
    inv_staged[staged] = np.arange(N_NODES)
    used[staged] = True
    dinv2all = np.where(used, 1.0 / deg[inv_staged], 0.0).astype(np.float32)
    dinv2col = dinv2all.reshape(C, TILES, P).transpose(0, 2, 1)  # [C,P,TILES]
    xtl = xs.reshape(C, TILES, P, D_PAD).transpose(0, 2, 1, 3).reshape(
        C, P, TILES * D_PAD)

    # ---- aggregation blocks (REAL edges only): per (core, tile).
    # Two overlapping int16 windows: w1=[0,HALF), w2=[B2V,NALL).  Sources in
    # [B2V,HALF) are flexible; assign just enough of them to w1 to pad each
    # (core,tile) w1 count to a full multiple of 128 (shared block count K1).
    norm = norm[:N_EDGES]
    ec = nodec[dst]
    et = nodet[dst]
    eslot = nodesl[dst]
    esrc = staged[src]
    cls3 = np.where(esrc < B2V, 0, np.where(esrc < HALF, 1, 2))
    key3 = (ec * TILES + et) * 3 + cls3
    cnt3 = np.bincount(key3, minlength=C * TILES * 3).reshape(C, TILES, 3)
    n1, nf, n2 = cnt3[:, :, 0], cnt3[:, :, 1], cnt3[:, :, 2]
    K1 = -(-n1.max(axis=0) // P)                     # [TILES]
    a = np.minimum(np.maximum(K1[None, :] * P - n1, 0), nf)   # flex -> w1
    ehi_cnt1 = n1 + a
    ehi_cnt2 = n2 + nf - a
    BL = np.maximum(-(-ehi_cnt1.max(axis=0) // P), K1)
    BH = -(-ehi_cnt2.max(axis=0) // P)
    # per-edge window: strict by cls3; flex edges -> w1 iff flex-rank < a[c,t]
    ord3 = np.argsort(key3, kind="stable")
    st3 = np.zeros(C * TILES * 3 + 1, np.int64)
    st3[1:] = np.cumsum(cnt3.reshape(-1))
    rank3 = np.arange(N_EDGES) - st3[key3[ord3]]
    ehi_o = np.empty(N_EDGES, np.int64)
    c3o = cls3[ord3]
    ehi_o[c3o == 0] = 0
    ehi_o[c3o == 2] = 1
    fm = c3o == 1
    ehi_o[fm] = (rank3[fm] >= a[ec[ord3][fm], et[ord3][fm]]).astype(np.int64)
    ehi = np.empty(N_EDGES, np.int64)
    ehi[ord3] = ehi_o
    key = (ec * TILES + et) * 2 + ehi
    order = np.argsort(key, kind="stable")
    cnt = np.bincount(key, minlength=C * TILES * 2).reshape(C, TILES, 2)

    groups = [list(range(i, min(i + GT, TILES))) for i in range(0, TILES, GT)]
    bufpos_lo = np.zeros(TILES, np.int64)
    bufpos_hi = np.zeros(TILES, np.int64)
    group_info = []   # per group: (g0pos, nbg, calls=[(cls, bufoff, nb)])
    pos = 0
    for tl in groups:
        g0 = pos
        for t in tl:
            bufpos_lo[t] = pos
            pos += BL[t]
        lo_nb = pos - g0
        for t in tl:
            bufpos_hi[t] = pos
            pos += BH[t]
        hi_nb = pos - g0 - lo_nb
        calls = []
        for cls, coff, cnb in ((0, 0, lo_nb), (1, lo_nb, hi_nb)):
            o = 0
            while o < cnb:
                nb = min(MAXB, cnb - o)
                calls.append((cls, coff + o, nb))
                o += nb
        group_info.append((g0, pos - g0, calls))
    SB = pos

    start = np.zeros(C * TILES * 2 + 1, np.int64)
    start[1:] = np.cumsum(cnt.reshape(-1))
    rank = np.arange(len(order)) - start[key[order]]
    base = np.where(ehi[order] == 0, bufpos_lo[et[order]], bufpos_hi[et[order]])
    col = base + rank // P
    pp = rank % P
    cs = ec[order]
    flat = col * P + pp

    meta = np.zeros((C, P, 2 * SB), dtype=np.float32)
    meta[cs, pp, 2 * col] = eslot[order].astype(np.float32)
    meta[cs, pp, 2 * col + 1] = norm[order]

    aggi = np.zeros((C, SB * P), dtype=np.int64)
    shifted = esrc[order] - ehi[order] * B2V
    edge_pos = np.full((C, SB * P), -1, np.int64)
    s01t = np.zeros((C, P, SB * P), dtype=np.float16)
    for c in range(C):
        m = cs == c
        aggi[c][flat[m]] = shifted[m]
        edge_pos[c][flat[m]] = order[m]
        s01t[c][eslot[order][m], flat[m]] = 1.0
    widx = np.stack([_wrap16(aggi[c]) for c in range(C)])

    # hi-ness per buffer block (to undo the -HALF shift when pre-gathering)
    hi_blocks = np.zeros(SB, np.int64)
    for g0, nbg, calls in group_info:
        for cls, boff, nb in calls:
            if cls == 1:
                hi_blocks[g0 + boff:g0 + boff + nb] = 1

    # host pre-gathered layer-1 stream, already in SBUF layout:
    # xgath[p, b*640:(b+1)*640] = xs row of buffer slot (b, p)
    xgath = []
    for c in range(C):
        rows = aggi[c].reshape(SB, P) + hi_blocks[:, None] * B2V
        g = xs[rows.reshape(-1)].reshape(SB, P, D_PAD)
        xgath.append(np.ascontiguousarray(
            g.transpose(1, 0, 2).reshape(P, SB * D_PAD)))

    iota = np.broadcast_to(np.arange(P, dtype=np.float16), (P, P)).copy()
    ident = np.eye(P, dtype=np.float16)

    shared = {
        "w1": np.ascontiguousarray(W1.astype(np.float16)),
        "w2": np.ascontiguousarray(W2.astype(np.float16)),
        "b1c": np.ascontiguousarray(b1.astype(np.float32).reshape(D_H, 1)),
        "b2r": np.ascontiguousarray(
            np.broadcast_to(b2.astype(np.float32), (P, D_OUT))),
        "iota": iota,
        "ident": ident,
    }
    in_maps = []
    for c in range(C):
        m = dict(shared)
        m["xgath"] = xgath[c]
        m["meta"] = np.ascontiguousarray(meta[c])
        m["widx"] = np.ascontiguousarray(widx[c])
        m["s01t"] = np.ascontiguousarray(s01t[c])
        m["xtl"] = np.ascontiguousarray(xtl[c])
        m["dinv2"] = np.ascontiguousarray(dinv2col[c])
        in_maps.append(m)

    plan = dict(
        SB=SB, groups=groups, group_info=group_info,
        BL=[int(v) for v in BL], BH=[int(v) for v in BH],
        bufpos_lo=bufpos_lo, bufpos_hi=bufpos_hi,
    )
    return in_maps, plan, edge_pos


# ---------------------------------------------------------------- device program
def _build(plan, ph=9):
    SB = plan["SB"]
    groups, group_info = plan["groups"], plan["group_info"]
    BL, BH = plan["BL"], plan["BH"]
    bufpos_lo, bufpos_hi = plan["bufpos_lo"], plan["bufpos_hi"]

    nc = bacc.Bacc("TRN2", target_bir_lowering=False, debug=False,
                   enable_asserts=False, num_devices=C)

    xgath_d = nc.dram_tensor("xgath", [P, SB * D_PAD], F16,
                             kind="ExternalInput")
    w1 = nc.dram_tensor("w1", [D_IN, D_H], F16, kind="ExternalInput")
    w2 = nc.dram_tensor("w2", [D_H, D_OUT], F16, kind="ExternalInput")
    b1c = nc.dram_tensor("b1c", [D_H, 1], F32, kind="ExternalInput")
    b2r = nc.dram_tensor("b2r", [P, D_OUT], F32, kind="ExternalInput")
    iota_d = nc.dram_tensor("iota", [P, P], F16, kind="ExternalInput")
    ident_d = nc.dram_tensor("ident", [P, P], F16, kind="ExternalInput")
    meta_d = nc.dram_tensor("meta", [P, 2 * SB], F32, kind="ExternalInput")
    widx_d = nc.dram_tensor("widx", [P, SB * 8], I16, kind="ExternalInput")
    s01t_d = nc.dram_tensor("s01t", [P, SB * P], F16, kind="ExternalInput")
    xtl_d = nc.dram_tensor("xtl", [P, TILES * D_PAD], F16,
                           kind="ExternalInput")
    dinv2_d = nc.dram_tensor("dinv2", [P, TILES], F32, kind="ExternalInput")
    logits_d = nc.dram_tensor("logits", [P, SB], F32, kind="ExternalOutput")

    rg = [list(range(C))]
    nbg_max = max(gi[1] for gi in group_info)

    with tile.TileContext(nc) as tc:
        with (
            tc.tile_pool(name="const", bufs=1) as constp,
            tc.tile_pool(name="meta", bufs=1) as metap,
            tc.tile_pool(name="sblk", bufs=6) as sp,
            tc.tile_pool(name="xagg", bufs=2) as xaggp,
            tc.tile_pool(name="kxn", bufs=2) as kxnp,
            tc.tile_pool(name="h1r", bufs=2) as h1rp,
            tc.tile_pool(name="h2s", bufs=2) as h2sp,
            tc.tile_pool(name="zz", bufs=3) as zp,
            tc.tile_pool(name="pacc", bufs=2, space="PSUM") as pacc,
            tc.tile_pool(name="ptp", bufs=2, space="PSUM") as ptp,
            tc.tile_pool(name="ph", bufs=2, space="PSUM") as php,
            tc.tile_pool(name="dram", bufs=1, space="DRAM") as dramp,
        ):
            # ---- constants / tables
            w1sb = []
            for k, (k0, kw) in enumerate(KCH):
                t = constp.tile([kw, D_H], F16, name=f"w1sb{k}", tag=f"w1sb{k}")
                nc.sync.dma_start(out=t[:], in_=w1[k0:k0 + kw, :])
                w1sb.append(t)
            w2sb, b1sb = [], []
            for m, (m0, mw) in enumerate(MCH):
                t = constp.tile([mw, D_OUT], F16, name=f"w2sb{m}", tag=f"w2sb{m}")
                nc.sync.dma_start(out=t[:], in_=w2[m0:m0 + mw, :])
                w2sb.append(t)
                bt = constp.tile([mw, 1], F32, name=f"b1sb{m}", tag=f"b1sb{m}")
                nc.sync.dma_start(out=bt[:], in_=b1c[m0:m0 + mw, :])
                b1sb.append(bt)
            b2sb = constp.tile([P, D_OUT], F32, name="b2sb", tag="b2sb")
            nc.sync.dma_start(out=b2sb[:], in_=b2r[:, :])
            iot = constp.tile([P, P], F16, name="iot", tag="iot")
            nc.sync.dma_start(out=iot[:], in_=iota_d[:, :])
            idn = constp.tile([P, P], F16, name="idn", tag="idn")
            nc.sync.dma_start(out=idn[:], in_=ident_d[:, :])
            meta_all = metap.tile([P, 2 * SB], F32, name="meta_all",
                                  tag="meta_all")
            nc.sync.dma_start(out=meta_all[:], in_=meta_d[:, :])
            widx_all = metap.tile([P, SB * 8], I16, name="widx_all",
                                  tag="widx_all")
            nc.sync.dma_start(out=widx_all[:], in_=widx_d[:, :])
            zkeep = metap.tile([P, TILES * D_OUT], F16, name="zkeep",
                               tag="zkeep")
            h2keep = metap.tile([P, TILES * D_OUT], F16, name="h2keep",
                                tag="h2keep")
            dinv2sb = metap.tile([P, TILES], F32, name="dinv2sb",
                                 tag="dinv2sb")
            nc.sync.dma_start(out=dinv2sb[:], in_=dinv2_d[:, :])

            h2loc = dramp.tile([NPAD, Z_PAD], F16, name="h2loc", tag="h2loc")
            h2full = dramp.tile([NALL, Z_PAD], F16, name="h2full",
                                tag="h2full", addr_space="Shared")
            zloc = dramp.tile([NPAD, Z_PAD], F16, name="zloc", tag="zloc")
            zfull = dramp.tile([NALL, Z_PAD], F16, name="zfull",
                               tag="zfull", addr_space="Shared")

            def build_s(o, eng=nc.vector):
                s_t = sp.tile([P, P], F16, name="s_t", tag="s_t")
                eng.tensor_scalar(
                    out=s_t[:], in0=iot[:],
                    scalar1=meta_all[:, 2 * o:2 * o + 1],
                    scalar2=meta_all[:, 2 * o + 1:2 * o + 2],
                    op0=mybir.AluOpType.is_equal,
                    op1=mybir.AluOpType.mult)
                return s_t

            def tile_runs(t):
                """[(first_bufpos, n)] runs of tile t's blocks."""
                runs = []
                if BL[t]:
                    runs.append((int(bufpos_lo[t]), BL[t]))
                if BH[t]:
                    runs.append((int(bufpos_hi[t]), BH[t]))
                return runs

            def tile_blocks(t):
                out = []
                for r0, n in tile_runs(t):
                    out.extend(range(r0, r0 + n))
                return [(pb, i == 0, i == len(out) - 1)
                        for i, pb in enumerate(out)]

            def issue_gathers(dst_tile, g0, calls, table, elem):
                for cls, boff, nb in calls:
                    view = table[B2V:, :] if cls else table[:, :]
                    n = nb * P
                    c0 = (g0 + boff) * 8
                    nc.gpsimd.dma_gather(
                        dst_tile[:, boff * elem:(boff + nb) * elem]
                        .rearrange("p (b e) -> p b e", e=elem),
                        view, widx_all[:, c0:c0 + n // 16], n, n, elem)

            # ---- phase A: L1 aggregate + GEMMs (static pre-gathered stream)
            with tc.tile_pool(name="xg", bufs=2) as xgp, \
                 tc.tile_pool(name="xt", bufs=2) as xtp:
                for g, tl in enumerate(groups):
                    g0, nbg, calls = group_info[g]
                    gw = len(tl) * P
                    xg = xgp.tile([P, nbg_max * D_PAD], F16, name="xg",
                                  tag="xg")
                    nc.sync.dma_start(
                        out=xg[:, 0:nbg * D_PAD],
                        in_=xgath_d[:, g0 * D_PAD:(g0 + nbg) * D_PAD])
                    xtg = xtp.tile([P, GT * D_PAD], F16, name="xtg", tag="xtg")
                    nc.sync.dma_start(
                        out=xtg[:, 0:len(tl) * D_PAD],
                        in_=xtl_d[:, tl[0] * D_PAD:(tl[-1] + 1) * D_PAD])
                    kxn = [kxnp.tile([P, gw], F16, name=f"kxn{k}", tag=f"kxn{k}")
                           for k in range(len(KCH))]
                    for j, t in enumerate(tl):
                        acc = pacc.tile([P, D_IN], F32, name="acc", tag="acc")
                        for pb, first, last in tile_blocks(t):
                            s_t = build_s(pb)
                            rb = pb - g0
                            nc.tensor.matmul(
                                acc[:, 0:512], lhsT=s_t[:],
                                rhs=xg[:, rb * D_PAD:rb * D_PAD + 512],
                                start=first, stop=last)
                            nc.tensor.matmul(
                                acc[:, 512:D_IN], lhsT=s_t[:],
                                rhs=xg[:, rb * D_PAD + 512:rb * D_PAD + D_IN],
                                start=first, stop=last)
                        xaggsb = xaggp.tile([P, D_IN], F16, name="xaggsb",
                                            tag="xaggsb")
                        xin = xtg[:, j * D_PAD:j * D_PAD + D_IN]
                        if tile_blocks(t):
                            nc.vector.scalar_tensor_tensor(
                                out=xaggsb[:], in0=xin,
                                scalar=dinv2sb[:, t:t + 1], in1=acc[:],
                                op0=mybir.AluOpType.mult,
                                op1=mybir.AluOpType.add)
                        else:
                            nc.vector.tensor_scalar(
                                out=xaggsb[:], in0=xin,
                                scalar1=dinv2sb[:, t:t + 1], scalar2=None,
                                op0=mybir.AluOpType.mult,
                                op1=mybir.AluOpType.bypass)
                        for k, (k0, kw) in enumerate(KCH):
                            tp = ptp.tile([P, P], F16, name="tp", tag="tp")
                            nc.tensor.transpose(out=tp[:kw, :],
                                                in_=xaggsb[:, k0:k0 + kw],
                                                identity=idn[:])
                            nc.vector.tensor_copy(
                                out=kxn[k][:kw, j * P:(j + 1) * P],
                                in_=tp[:kw, :])
                    h1r = [h1rp.tile([mw, gw], F16, name=f"h1r{m}",
                                     tag=f"h1r{m}")
                           for m, (m0, mw) in enumerate(MCH)]
                    for m, (m0, mw) in enumerate(MCH):
                        hp = php.tile([P, gw], F32, name="hp", tag="hp")
                        for k, (k0, kw) in enumerate(KCH):
                            nc.tensor.matmul(hp[:mw, :],
                                             lhsT=w1sb[k][:, m0:m0 + mw],
                                             rhs=kxn[k][:kw, :],
                                             start=(k == 0),
                                             stop=(k == len(KCH) - 1))
                        nc.scalar.activation(
                            out=h1r[m][:], in_=hp[:mw, :],
                            func=mybir.ActivationFunctionType.Relu,
                            bias=b1sb[m][:], scale=1.0)
                    h2p = php.tile([P, gw], F32, name="h2p", tag="hp")
                    for m, (m0, mw) in enumerate(MCH):
                        nc.tensor.matmul(h2p[:D_OUT, :], lhsT=w2sb[m][:],
                                         rhs=h1r[m][:],
                                         start=(m == 0),
                                         stop=(m == len(MCH) - 1))
                    h2sb = h2sp.tile([D_OUT, gw], F16, name="h2sb", tag="h2sb")
                    nc.scalar.copy(out=h2sb[:], in_=h2p[:D_OUT, :])
                    for j, t in enumerate(tl):
                        tp2 = ptp.tile([P, P], F16, name="tp2", tag="tp")
                        nc.tensor.transpose(out=tp2[:, :D_OUT],
                                            in_=h2sb[:, j * P:(j + 1) * P],
                                            identity=idn[:D_OUT, :D_OUT])
                        h2row = zp.tile([P, D_OUT], F16, name="h2row",
                                        tag="h2row")
                        nc.vector.tensor_copy(out=h2row[:], in_=tp2[:, :D_OUT])
                        nc.vector.tensor_copy(
                            out=h2keep[:, t * D_OUT:(t + 1) * D_OUT],
                            in_=h2row[:])
                        nc.sync.dma_start(
                            out=h2loc[t * P:(t + 1) * P, 0:D_OUT],
                            in_=h2row[:])

            if ph >= 2:
                nc.gpsimd.collective_compute(
                    "AllGather", mybir.AluOpType.bypass, replica_groups=rg,
                    ins=[h2loc[:].opt()], outs=[h2full[:].opt()])

            # ---- phase C: L2 aggregate
            with tc.tile_pool(name="mg", bufs=3) as mgp:
                for g, tl in (list(enumerate(groups)) if ph >= 3 else []):
                    g0, nbg, calls = group_info[g]
                    mg = mgp.tile([P, nbg_max * Z_PAD], F16, name="mg",
                                  tag="mg")
                    issue_gathers(mg, g0, calls, h2full, Z_PAD)
                    for t in tl:
                        acc2 = ptp.tile([P, D_OUT], F32, name="acc2", tag="tp")
                        for pb, first, last in tile_blocks(t):
                            rb = pb - g0
                            s_t = build_s(pb)
                            nc.tensor.matmul(
                                acc2[:], lhsT=s_t[:],
                                rhs=mg[:, rb * Z_PAD:rb * Z_PAD + D_OUT],
                                start=first, stop=last)
                        zsb = zp.tile([P, Z_PAD], F16, name="zsb", tag="zsb")
                        nc.vector.memset(zsb[:, D_OUT:Z_PAD], 0.0)
                        ztmp = zp.tile([P, D_OUT], F32, name="ztmp",
                                       tag="ztmp")
                        h2in = h2keep[:, t * D_OUT:(t + 1) * D_OUT]
                        if tile_blocks(t):
                            nc.vector.scalar_tensor_tensor(
                                out=ztmp[:], in0=h2in,
                                scalar=dinv2sb[:, t:t + 1], in1=acc2[:],
                                op0=mybir.AluOpType.mult,
                                op1=mybir.AluOpType.add)
                        else:
                            nc.vector.tensor_scalar(
                                out=ztmp[:], in0=h2in,
                                scalar1=dinv2sb[:, t:t + 1], scalar2=None,
                                op0=mybir.AluOpType.mult,
                                op1=mybir.AluOpType.bypass)
                        nc.vector.tensor_add(out=zsb[:, 0:D_OUT], in0=ztmp[:],
                                             in1=b2sb[:])
                        nc.vector.tensor_copy(
                            out=zkeep[:, t * D_OUT:(t + 1) * D_OUT],
                            in_=zsb[:, 0:D_OUT])
                        nc.sync.dma_start(
                            out=zloc[t * P:(t + 1) * P, :], in_=zsb[:])

            if ph >= 4:
                nc.gpsimd.collective_compute(
                    "AllGather", mybir.AluOpType.bypass, replica_groups=rg,
                    ins=[zloc[:].opt()], outs=[zfull[:].opt()])

            # ---- phase E: decoder in aggregation order
            with (
                tc.tile_pool(name="zsg", bufs=3) as zsgp,
                tc.tile_pool(name="s01", bufs=2) as s01p,
                tc.tile_pool(name="zds", bufs=3) as zdsp,
                tc.tile_pool(name="lac", bufs=1) as lacp,
            ):
                lacc = lacp.tile([P, SB], F32, name="lacc", tag="lacc")
                if ph < 5:
                    nc.gpsimd.memset(lacc[:], 0.0)
                def dec_chunk(g0, zsg, s01, t, r0, o, ch):
                    zdp = php.tile([P, 512], F32, name="zdp", tag="hp")
                    for i in range(ch):
                        rb = r0 + o + i - g0
                        nc.tensor.matmul(
                            zdp[:, i * D_OUT:(i + 1) * D_OUT],
                            lhsT=s01[:, rb * P:(rb + 1) * P],
                            rhs=zkeep[:, t * D_OUT:(t + 1) * D_OUT],
                            start=True, stop=True)
                    zds = zdsp.tile([P, MAXB * Z_PAD], F16, name="zds",
                                    tag="zds")
                    nc.vector.memset(zds[:], 0.0)
                    for i in range(ch):
                        nc.scalar.copy(
                            out=zds[:, i * Z_PAD:i * Z_PAD + D_OUT],
                            in_=zdp[:, i * D_OUT:(i + 1) * D_OUT])
                    prod = zdsp.tile([P, MAXB * Z_PAD], F16, name="prod",
                                     tag="prod")
                    c0 = (r0 + o - g0) * Z_PAD
                    nc.vector.tensor_mul(
                        out=prod[:, 0:ch * Z_PAD],
                        in0=zsg[:, c0:c0 + ch * Z_PAD],
                        in1=zds[:, 0:ch * Z_PAD])
                    nc.vector.reduce_sum(
                        out=lacc[:, r0 + o:r0 + o + ch],
                        in_=prod[:, 0:ch * Z_PAD]
                        .rearrange("p (b e) -> p b e", e=Z_PAD),
                        axis=mybir.AxisListType.X)

                def dec_group(g, tl):
                    g0, nbg, calls = group_info[g]
                    zsg = zsgp.tile([P, nbg_max * Z_PAD], F16, name="zsg",
                                    tag="zsg")
                    issue_gathers(zsg, g0, calls, zfull, Z_PAD)
                    s01 = s01p.tile([P, nbg_max * P], F16, name="s01",
                                    tag="s01")
                    nc.sync.dma_start(
                        out=s01[:, 0:nbg * P],
                        in_=s01t_d[:, g0 * P:(g0 + nbg) * P])
                    for t in tl:
                        for r0, rn in tile_runs(t):
                            o = 0
                            while o < rn:
                                ch = min(MAXB, rn - o)
                                dec_chunk(g0, zsg, s01, t, r0, o, ch)
                                o += ch

                for g, tl in (list(enumerate(groups)) if ph >= 5 else []):
                    dec_group(g, tl)
                nc.sync.dma_start(out=logits_d[:, :], in_=lacc[:])

    nc.compile()
    return nc


# ---------------------------------------------------------------- entry point
_CACHE = {}


def kernel(x, edge_index, W1, b1, W2, b2):
    x = np.asarray(x)
    edge_index = np.asarray(edge_index)
    in_maps, plan, edge_pos = _preprocess(
        x, edge_index, np.asarray(W1), np.asarray(b1),
        np.asarray(W2), np.asarray(b2))
    key = (plan["SB"], tuple(plan["BL"]), tuple(plan["BH"]))
    if key not in _CACHE:
        _CACHE[key] = _build(plan)
    nc = _CACHE[key]
    res = bass_utils.run_bass_kernel_spmd(nc, in_maps, core_ids=list(range(C)))
    out = np.empty(N_EDGES, dtype=np.float32)
    for c in range(C):
        lg = res.results[c]["logits"]           # [P, SB]
        flat = lg.T.reshape(-1)                 # position pb*128+p
        ok = edge_pos[c] >= 0
        out[edge_pos[c][ok]] = flat[ok]
    return out


# revision 15
# speedup vs baseline: 1.1832x; 1.1832x over previous
"""Trainium2 Bass kernel for a 2-layer GCN encoder + edge dot-product decoder.

Math (matches the PyG-style reference):
    deg  = in-degree(dst)+1 (self loops), dinv = rsqrt(deg)
    A~[d,s] = dinv[s]*dinv[d] over edges+self-loops
    H1 = (A~ @ X) @ W1 + b1          (aggregate-first ordering)
    Z  = (A~ @ relu(H1) @ W2) + b2
    logits[e] = <Z[src_e], Z[dst_e]>

Distribution over 8 NeuronCores: nodes sharded via LPT balancing (staged
order), edges partitioned by destination owner, weights replicated.

Key measured constraint: every dynamic row-gather on the SWDGE path costs
~8.9ns/row regardless of batch/row size, serialized on GpSimd.  Design:
  * Layer-1 gather of x[src] rows is STATIC (indices known on host), so the
    host pre-gathers the edge stream into xgath (SBUF layout) and the device
    streams it with static DMA -- zero SWDGE.
  * Layer-2 must gather h2[src] (dynamic, 256B rows) -- batched dma_gather,
    <=1024 rows/call, lo/hi split for signed int16 indices.
  * The decoder runs in the SAME aggregation block structure: z[src] reuses
    the same index table (gathered from zfull), z[dst] is selected from the
    locally-kept z tile via host-shipped one-hot S01T matmuls (zero SWDGE),
    and the dot-products are batched mul + strided 3-D reduce.
The segment-sum scatter is a [128e x 128slot] matrix S (iota is_equal slot
* norm) applied on the Tensor engine with PSUM accumulation.
"""

import os

if os.environ.get("JAX_PLATFORMS") == "cpu":
    os.environ.pop("JAX_PLATFORMS")

import numpy as np

from concourse import bass, bacc, mybir, bass_utils
import concourse.tile as tile

# ---------------------------------------------------------------- sizes
N_NODES = 50000
N_EDGES = 400000
D_IN, D_H, D_OUT = 600, 628, 64
D_PAD = 640                     # x rows padded to 640 f16 = 1280 B
Z_PAD = 128                     # h2/z rows padded to 128 f16 = 256 B
C = 8
P = 128
HALF = 32768                    # int16 signed gather-index window size
B2V = 50176 - HALF              # second window base: [B2V, NALL), idx-B2V
TH = 24                         # tiles in the first AllGather half
MAXB = 8                        # blocks per dma_gather call (8*128=1024 idx)
GT = 4                          # tiles per GEMM group

F16 = mybir.dt.float16
F32 = mybir.dt.float32
I16 = mybir.dt.int16

NPC = N_NODES // C              # 6250
TILES = -(-NPC // P)            # 49
NPAD = TILES * P                # 6272
NALL = C * NPAD                 # 50176


def _chunks(total, step=128):
    out, o = [], 0
    while o < total:
        w = min(step, total - o)
        out.append((o, w))
        o += w
    return out


KCH = _chunks(D_IN)
MCH = _chunks(D_H)


def _wrap16(idx):
    """1-D int array (len % 128 == 0) -> [128, len/16] int16 wrapped layout:
    position i lives at (partition i%16, col i//16), tiled to 128 rows."""
    n = len(idx)
    t = np.asarray(idx, np.int64).reshape(n // 16, 16).T
    t = t.astype(np.uint16).view(np.int16)
    return np.tile(t, (8, 1))


# ---------------------------------------------------------------- host preprocessing
def _assign_nodes(d_all, N, tiles):
    """LPT-assign nodes to C*tiles buckets of <=128 slots, minimizing the
    max per-bucket edge count."""
    import heapq
    w = np.bincount(d_all, minlength=N)
    nb = C * tiles
    heap = [(0, b) for b in range(nb)]
    heapq.heapify(heap)
    cnt = np.zeros(nb, np.int64)
    nodec = np.empty(N, np.int64)
    nodet = np.empty(N, np.int64)
    nodesl = np.empty(N, np.int64)
    for n in np.argsort(-w, kind="stable"):
        while True:
            wt, b = heapq.heappop(heap)
            if cnt[b] < P:
                break
        nodec[n] = b // tiles
        nodet[n] = b % tiles
        nodesl[n] = cnt[b]
        cnt[b] += 1
        if cnt[b] < P:
            heapq.heappush(heap, (wt + int(w[n]), b))
    return nodec, nodet, nodesl


def _preprocess(x, edge_index, W1, b1, W2, b2):
    src = edge_index[0].astype(np.int64)
    dst = edge_index[1].astype(np.int64)
    loop = np.arange(N_NODES, dtype=np.int64)
    s_all = np.concatenate([src, loop])
    d_all = np.concatenate([dst, loop])
    deg = np.bincount(d_all, minlength=N_NODES).astype(np.float64)
    dinv = 1.0 / np.sqrt(deg)
    norm = (dinv[s_all] * dinv[d_all]).astype(np.float32)

    nodec, nodet, nodesl = _assign_nodes(dst, N_NODES, TILES)
    staged = nodec * NPAD + nodet * P + nodesl

    xs = np.zeros((NALL, D_PAD), dtype=np.float16)
    xs[staged, :D_IN] = x.astype(np.float16)

    # per-core staged x rows + self-loop weights (self-loops are folded into
    # the aggregation output instead of occupying block slots)
    inv_staged = np.zeros(NALL, np.int64)
    used = np.zeros(NALL, bool)
    inv_staged[staged] = np.arange(N_NODES)
    used[staged] = True
    dinv2all = np.where(used, 1.0 / deg[inv_staged], 0.0).astype(np.float32)
    dinv2col = dinv2all.reshape(C, TILES, P).transpose(0, 2, 1)  # [C,P,TILES]
    xtl = xs.reshape(C, TILES, P, D_PAD).transpose(0, 2, 1, 3).reshape(
        C, P, TILES * D_PAD)

    # ---- aggregation blocks (REAL edges only): per (core, tile).
    # Two overlapping int16 windows: w1=[0,HALF), w2=[B2V,NALL).  Sources in
    # [B2V,HALF) are flexible; assign just enough of them to w1 to pad each
    # (core,tile) w1 count to a full multiple of 128 (shared block count K1).
    norm = norm[:N_EDGES]
    ec = nodec[dst]
    et = nodet[dst]
    eslot = nodesl[dst]
    esrc = staged[src]
    cls3 = np.where(esrc < B2V, 0, np.where(esrc < HALF, 1, 2))
    key3 = (ec * TILES + et) * 3 + cls3
    cnt3 = np.bincount(key3, minlength=C * TILES * 3).reshape(C, TILES, 3)
    n1, nf, n2 = cnt3[:, :, 0], cnt3[:, :, 1], cnt3[:, :, 2]
    K1 = -(-n1.max(axis=0) // P)                     # [TILES]
    # raise K1 within flex headroom so each GT-group's lo-block sum is a
    # multiple of MAXB (full 1024-row gather calls, fewer call boundaries)
    Kmax = np.minimum((n1 + nf).min(axis=0) // P, -(-(n1 + nf + n2).max(axis=0) // P))
    for i in range(0, TILES, GT):
        tl = list(range(i, min(i + GT, TILES)))
        need = (-int(K1[tl].sum())) % MAXB
        for t in tl:
            room = int(Kmax[t] - K1[t])
            add = min(room, need)
            K1[t] += add
            need -= add
            if need == 0:
                break
    a = np.minimum(np.maximum(K1[None, :] * P - n1, 0), nf)   # flex -> w1
    ehi_cnt1 = n1 + a
    ehi_cnt2 = n2 + nf - a
    BL = np.maximum(-(-ehi_cnt1.max(axis=0) // P), K1)
    BH = -(-ehi_cnt2.max(axis=0) // P)
    # per-edge window: strict by cls3; flex edges -> w1 iff flex-rank < a[c,t]
    ord3 = np.argsort(key3, kind="stable")
    st3 = np.zeros(C * TILES * 3 + 1, np.int64)
    st3[1:] = np.cumsum(cnt3.reshape(-1))
    rank3 = np.arange(N_EDGES) - st3[key3[ord3]]
    ehi_o = np.empty(N_EDGES, np.int64)
    c3o = cls3[ord3]
    ehi_o[c3o == 0] = 0
    ehi_o[c3o == 2] = 1
    fm = c3o == 1
    ehi_o[fm] = (rank3[fm] >= a[ec[ord3][fm], et[ord3][fm]]).astype(np.int64)
    ehi = np.empty(N_EDGES, np.int64)
    ehi[ord3] = ehi_o
    key = (ec * TILES + et) * 2 + ehi
    order = np.argsort(key, kind="stable")
    cnt = np.bincount(key, minlength=C * TILES * 2).reshape(C, TILES, 2)

    groups = [list(range(i, min(i + GT, TILES))) for i in range(0, TILES, GT)]
    bufpos_lo = np.zeros(TILES, np.int64)
    bufpos_hi = np.zeros(TILES, np.int64)
    group_info = []   # per group: (g0pos, nbg, calls=[(cls, bufoff, nb)])
    pos = 0
    for tl in groups:
        g0 = pos
        for t in tl:
            bufpos_lo[t] = pos
            pos += BL[t]
        lo_nb = pos - g0
        for t in tl:
            bufpos_hi[t] = pos
            pos += BH[t]
        hi_nb = pos - g0 - lo_nb
        calls = []
        for cls, coff, cnb in ((0, 0, lo_nb), (1, lo_nb, hi_nb)):
            o = 0
            while o < cnb:
                nb = min(MAXB, cnb - o)
                calls.append((cls, coff + o, nb))
                o += nb
        group_info.append((g0, pos - g0, calls))
    SB = pos

    start = np.zeros(C * TILES * 2 + 1, np.int64)
    start[1:] = np.cumsum(cnt.reshape(-1))
    rank = np.arange(len(order)) - start[key[order]]
    base = np.where(ehi[order] == 0, bufpos_lo[et[order]], bufpos_hi[et[order]])
    col = base + rank // P
    pp = rank % P
    cs = ec[order]
    flat = col * P + pp

    meta = np.zeros((C, P, 2 * SB), dtype=np.float32)
    meta[cs, pp, 2 * col] = eslot[order].astype(np.float32)
    meta[cs, pp, 2 * col + 1] = norm[order]

    aggi = np.zeros((C, SB * P), dtype=np.int64)
    shifted = esrc[order] - ehi[order] * B2V
    edge_pos = np.full((C, SB * P), -1, np.int64)
    s01t = np.zeros((C, P, SB * P), dtype=np.float16)
    for c in range(C):
        m = cs == c
        aggi[c][flat[m]] = shifted[m]
        edge_pos[c][flat[m]] = order[m]
        s01t[c][eslot[order][m], flat[m]] = 1.0
    widx = np.stack([_wrap16(aggi[c]) for c in range(C)])

    # hi-ness per buffer block (to undo the -HALF shift when pre-gathering)
    hi_blocks = np.zeros(SB, np.int64)
    for g0, nbg, calls in group_info:
        for cls, boff, nb in calls:
            if cls == 1:
                hi_blocks[g0 + boff:g0 + boff + nb] = 1

    # host pre-gathered layer-1 stream, already in SBUF layout:
    # xgath[p, b*640:(b+1)*640] = xs row of buffer slot (b, p)
    xgath = []
    for c in range(C):
        rows = aggi[c].reshape(SB, P) + hi_blocks[:, None] * B2V
        g = xs[rows.reshape(-1)].reshape(SB, P, D_PAD)
        xgath.append(np.ascontiguousarray(
            g.transpose(1, 0, 2).reshape(P, SB * D_PAD)))

    iota = np.broadcast_to(np.arange(P, dtype=np.float16), (P, P)).copy()
    ident = np.eye(P, dtype=np.float16)

    shared = {
        "w1": np.ascontiguousarray(W1.astype(np.float16)),
        "w2": np.ascontiguousarray(W2.astype(np.float16)),
        "b1c": np.ascontiguousarray(b1.astype(np.float32).reshape(D_H, 1)),
        "b2r": np.ascontiguousarray(
            np.broadcast_to(b2.astype(np.float32), (P, D_OUT))),
        "iota": iota,
        "ident": ident,
    }
    in_maps = []
    for c in range(C):
        m = dict(shared)
        m["xgath"] = xgath[c]
        m["meta"] = np.ascontiguousarray(meta[c])
        m["widx"] = np.ascontiguousarray(widx[c])
        m["s01t"] = np.ascontiguousarray(s01t[c])
        m["xtl"] = np.ascontiguousarray(xtl[c])
        m["dinv2"] = np.ascontiguousarray(dinv2col[c])
        in_maps.append(m)

    plan = dict(
        SB=SB, groups=groups, group_info=group_info,
        BL=[int(v) for v in BL], BH=[int(v) for v in BH],
        bufpos_lo=bufpos_lo, bufpos_hi=bufpos_hi,
    )
    return in_maps, plan, edge_pos


# ---------------------------------------------------------------- device program
def _build(plan, ph=9):
    SB = plan["SB"]
    groups, group_info = plan["groups"], plan["group_info"]
    BL, BH = plan["BL"], plan["BH"]
    bufpos_lo, bufpos_hi = plan["bufpos_lo"], plan["bufpos_hi"]

    nc = bacc.Bacc("TRN2", target_bir_lowering=False, debug=False,
                   enable_asserts=False, num_devices=C)

    xgath_d = nc.dram_tensor("xgath", [P, SB * D_PAD], F16,
                             kind="ExternalInput")
    w1 = nc.dram_tensor("w1", [D_IN, D_H], F16, kind="ExternalInput")
    w2 = nc.dram_tensor("w2", [D_H, D_OUT], F16, kind="ExternalInput")
    b1c = nc.dram_tensor("b1c", [D_H, 1], F32, kind="ExternalInput")
    b2r = nc.dram_tensor("b2r", [P, D_OUT], F32, kind="ExternalInput")
    iota_d = nc.dram_tensor("iota", [P, P], F16, kind="ExternalInput")
    ident_d = nc.dram_tensor("ident", [P, P], F16, kind="ExternalInput")
    meta_d = nc.dram_tensor("meta", [P, 2 * SB], F32, kind="ExternalInput")
    widx_d = nc.dram_tensor("widx", [P, SB * 8], I16, kind="ExternalInput")
    s01t_d = nc.dram_tensor("s01t", [P, SB * P], F16, kind="ExternalInput")
    xtl_d = nc.dram_tensor("xtl", [P, TILES * D_PAD], F16,
                           kind="ExternalInput")
    dinv2_d = nc.dram_tensor("dinv2", [P, TILES], F32, kind="ExternalInput")
    logits_d = nc.dram_tensor("logits", [P, SB], F32, kind="ExternalOutput")

    rg = [list(range(C))]
    nbg_max = max(gi[1] for gi in group_info)

    with tile.TileContext(nc) as tc:
        with (
            tc.tile_pool(name="const", bufs=1) as constp,
            tc.tile_pool(name="meta", bufs=1) as metap,
            tc.tile_pool(name="sblk", bufs=6) as sp,
            tc.tile_pool(name="xagg", bufs=2) as xaggp,
            tc.tile_pool(name="kxn", bufs=2) as kxnp,
            tc.tile_pool(name="h1r", bufs=2) as h1rp,
            tc.tile_pool(name="h2s", bufs=2) as h2sp,
            tc.tile_pool(name="zz", bufs=3) as zp,
            tc.tile_pool(name="pacc", bufs=2, space="PSUM") as pacc,
            tc.tile_pool(name="ptp", bufs=2, space="PSUM") as ptp,
            tc.tile_pool(name="ph", bufs=2, space="PSUM") as php,
            tc.tile_pool(name="dram", bufs=1, space="DRAM") as dramp,
        ):
            # ---- constants / tables
            w1sb = []
            for k, (k0, kw) in enumerate(KCH):
                t = constp.tile([kw, D_H], F16, name=f"w1sb{k}", tag=f"w1sb{k}")
                nc.sync.dma_start(out=t[:], in_=w1[k0:k0 + kw, :])
                w1sb.append(t)
            w2sb, b1sb = [], []
            for m, (m0, mw) in enumerate(MCH):
                t = constp.tile([mw, D_OUT], F16, name=f"w2sb{m}", tag=f"w2sb{m}")
                nc.sync.dma_start(out=t[:], in_=w2[m0:m0 + mw, :])
                w2sb.append(t)
                bt = constp.tile([mw, 1], F32, name=f"b1sb{m}", tag=f"b1sb{m}")
                nc.sync.dma_start(out=bt[:], in_=b1c[m0:m0 + mw, :])
                b1sb.append(bt)
            b2sb = constp.tile([P, D_OUT], F32, name="b2sb", tag="b2sb")
            nc.sync.dma_start(out=b2sb[:], in_=b2r[:, :])
            iot = constp.tile([P, P], F16, name="iot", tag="iot")
            nc.sync.dma_start(out=iot[:], in_=iota_d[:, :])
            idn = constp.tile([P, P], F16, name="idn", tag="idn")
            nc.sync.dma_start(out=idn[:], in_=ident_d[:, :])
            meta_all = metap.tile([P, 2 * SB], F32, name="meta_all",
                                  tag="meta_all")
            nc.sync.dma_start(out=meta_all[:], in_=meta_d[:, :])
            widx_all = metap.tile([P, SB * 8], I16, name="widx_all",
                                  tag="widx_all")
            nc.sync.dma_start(out=widx_all[:], in_=widx_d[:, :])
            zkeep = metap.tile([P, TILES * D_OUT], F16, name="zkeep",
                               tag="zkeep")
            h2keep = metap.tile([P, TILES * D_OUT], F16, name="h2keep",
                                tag="h2keep")
            dinv2sb = metap.tile([P, TILES], F32, name="dinv2sb",
                                 tag="dinv2sb")
            nc.sync.dma_start(out=dinv2sb[:], in_=dinv2_d[:, :])

            h2loc = dramp.tile([NPAD, Z_PAD], F16, name="h2loc", tag="h2loc")
            h2full = dramp.tile([NALL, Z_PAD], F16, name="h2full",
                                tag="h2full", addr_space="Shared")
            zloc = dramp.tile([NPAD, Z_PAD], F16, name="zloc", tag="zloc")
            zfull = dramp.tile([NALL, Z_PAD], F16, name="zfull",
                               tag="zfull", addr_space="Shared")

            def build_s(o, eng=nc.vector):
                s_t = sp.tile([P, P], F16, name="s_t", tag="s_t")
                eng.tensor_scalar(
                    out=s_t[:], in0=iot[:],
                    scalar1=meta_all[:, 2 * o:2 * o + 1],
                    scalar2=meta_all[:, 2 * o + 1:2 * o + 2],
                    op0=mybir.AluOpType.is_equal,
                    op1=mybir.AluOpType.mult)
                return s_t

            def tile_runs(t):
                """[(first_bufpos, n)] runs of tile t's blocks."""
                runs = []
                if BL[t]:
                    runs.append((int(bufpos_lo[t]), BL[t]))
                if BH[t]:
                    runs.append((int(bufpos_hi[t]), BH[t]))
                return runs

            def tile_blocks(t):
                out = []
                for r0, n in tile_runs(t):
                    out.extend(range(r0, r0 + n))
                return [(pb, i == 0, i == len(out) - 1)
                        for i, pb in enumerate(out)]

            def issue_gathers(dst_tile, g0, calls, table, elem):
                for cls, boff, nb in calls:
                    view = table[B2V:, :] if cls else table[:, :]
                    n = nb * P
                    c0 = (g0 + boff) * 8
                    nc.gpsimd.dma_gather(
                        dst_tile[:, boff * elem:(boff + nb) * elem]
                        .rearrange("p (b e) -> p b e", e=elem),
                        view, widx_all[:, c0:c0 + n // 16], n, n, elem)

            # ---- phase A: L1 aggregate + GEMMs (static pre-gathered stream)
            with tc.tile_pool(name="xg", bufs=2) as xgp, \
                 tc.tile_pool(name="xt", bufs=2) as xtp:
                for g, tl in enumerate(groups):
                    g0, nbg, calls = group_info[g]
                    gw = len(tl) * P
                    xg = xgp.tile([P, nbg_max * D_PAD], F16, name="xg",
                                  tag="xg")
                    nc.sync.dma_start(
                        out=xg[:, 0:nbg * D_PAD],
                        in_=xgath_d[:, g0 * D_PAD:(g0 + nbg) * D_PAD])
                    xtg = xtp.tile([P, GT * D_PAD], F16, name="xtg", tag="xtg")
                    nc.sync.dma_start(
                        out=xtg[:, 0:len(tl) * D_PAD],
                        in_=xtl_d[:, tl[0] * D_PAD:(tl[-1] + 1) * D_PAD])
                    kxn = [kxnp.tile([P, gw], F16, name=f"kxn{k}", tag=f"kxn{k}")
                           for k in range(len(KCH))]
                    for j, t in enumerate(tl):
                        acc = pacc.tile([P, D_IN], F32, name="acc", tag="acc")
                        for pb, first, last in tile_blocks(t):
                            s_t = build_s(pb)
                            rb = pb - g0
                            nc.tensor.matmul(
                                acc[:, 0:512], lhsT=s_t[:],
                                rhs=xg[:, rb * D_PAD:rb * D_PAD + 512],
                                start=first, stop=last)
                            nc.tensor.matmul(
                                acc[:, 512:D_IN], lhsT=s_t[:],
                                rhs=xg[:, rb * D_PAD + 512:rb * D_PAD + D_IN],
                                start=first, stop=last)
                        xaggsb = xaggp.tile([P, D_IN], F16, name="xaggsb",
                                            tag="xaggsb")
                        xin = xtg[:, j * D_PAD:j * D_PAD + D_IN]
                        if tile_blocks(t):
                            nc.vector.scalar_tensor_tensor(
                                out=xaggsb[:], in0=xin,
                                scalar=dinv2sb[:, t:t + 1], in1=acc[:],
                                op0=mybir.AluOpType.mult,
                                op1=mybir.AluOpType.add)
                        else:
                            nc.vector.tensor_scalar(
                                out=xaggsb[:], in0=xin,
                                scalar1=dinv2sb[:, t:t + 1], scalar2=None,
                                op0=mybir.AluOpType.mult,
                                op1=mybir.AluOpType.bypass)
                        for k, (k0, kw) in enumerate(KCH):
                            tp = ptp.tile([P, P], F16, name="tp", tag="tp")
                            nc.tensor.transpose(out=tp[:kw, :],
                                                in_=xaggsb[:, k0:k0 + kw],
                                                identity=idn[:])
                            nc.vector.tensor_copy(
                                out=kxn[k][:kw, j * P:(j + 1) * P],
                                in_=tp[:kw, :])
                    h1r = [h1rp.tile([mw, gw], F16, name=f"h1r{m}",
                                     tag=f"h1r{m}")
                           for m, (m0, mw) in enumerate(MCH)]
                    for m, (m0, mw) in enumerate(MCH):
                        hp = php.tile([P, gw], F32, name="hp", tag="hp")
                        for k, (k0, kw) in enumerate(KCH):
                            nc.tensor.matmul(hp[:mw, :],
                                             lhsT=w1sb[k][:, m0:m0 + mw],
                                             rhs=kxn[k][:kw, :],
                                             start=(k == 0),
                                             stop=(k == len(KCH) - 1))
                        nc.scalar.activation(
                            out=h1r[m][:], in_=hp[:mw, :],
                            func=mybir.ActivationFunctionType.Relu,
                            bias=b1sb[m][:], scale=1.0)
                    h2p = php.tile([P, gw], F32, name="h2p", tag="hp")
                    for m, (m0, mw) in enumerate(MCH):
                        nc.tensor.matmul(h2p[:D_OUT, :], lhsT=w2sb[m][:],
                                         rhs=h1r[m][:],
                                         start=(m == 0),
                                         stop=(m == len(MCH) - 1))
                    h2sb = h2sp.tile([D_OUT, gw], F16, name="h2sb", tag="h2sb")
                    nc.scalar.copy(out=h2sb[:], in_=h2p[:D_OUT, :])
                    for j, t in enumerate(tl):
                        tp2 = ptp.tile([P, P], F16, name="tp2", tag="tp")
                        nc.tensor.transpose(out=tp2[:, :D_OUT],
                                            in_=h2sb[:, j * P:(j + 1) * P],
                                            identity=idn[:D_OUT, :D_OUT])
                        h2row = zp.tile([P, D_OUT], F16, name="h2row",
                                        tag="h2row")
                        nc.vector.tensor_copy(out=h2row[:], in_=tp2[:, :D_OUT])
                        nc.vector.tensor_copy(
                            out=h2keep[:, t * D_OUT:(t + 1) * D_OUT],
                            in_=h2row[:])
                        nc.sync.dma_start(
                            out=h2loc[t * P:(t + 1) * P, 0:D_OUT],
                            in_=h2row[:])

            if ph >= 2:
                nc.gpsimd.collective_compute(
                    "AllGather", mybir.AluOpType.bypass, replica_groups=rg,
                    ins=[h2loc[:].opt()], outs=[h2full[:].opt()])

            # ---- phase C: L2 aggregate
            with tc.tile_pool(name="mg", bufs=3) as mgp:
                for g, tl in (list(enumerate(groups)) if ph >= 3 else []):
                    g0, nbg, calls = group_info[g]
                    mg = mgp.tile([P, nbg_max * Z_PAD], F16, name="mg",
                                  tag="mg")
                    issue_gathers(mg, g0, calls, h2full, Z_PAD)
                    for t in tl:
                        acc2 = ptp.tile([P, D_OUT], F32, name="acc2", tag="tp")
                        for pb, first, last in tile_blocks(t):
                            rb = pb - g0
                            s_t = build_s(pb)
                            nc.tensor.matmul(
                                acc2[:], lhsT=s_t[:],
                                rhs=mg[:, rb * Z_PAD:rb * Z_PAD + D_OUT],
                                start=first, stop=last)
                        zsb = zp.tile([P, Z_PAD], F16, name="zsb", tag="zsb")
                        nc.vector.memset(zsb[:, D_OUT:Z_PAD], 0.0)
                        ztmp = zp.tile([P, D_OUT], F32, name="ztmp",
                                       tag="ztmp")
                        h2in = h2keep[:, t * D_OUT:(t + 1) * D_OUT]
                        if tile_blocks(t):
                            nc.vector.scalar_tensor_tensor(
                                out=ztmp[:], in0=h2in,
                                scalar=dinv2sb[:, t:t + 1], in1=acc2[:],
                                op0=mybir.AluOpType.mult,
                                op1=mybir.AluOpType.add)
                        else:
                            nc.vector.tensor_scalar(
                                out=ztmp[:], in0=h2in,
                                scalar1=dinv2sb[:, t:t + 1], scalar2=None,
                                op0=mybir.AluOpType.mult,
                                op1=mybir.AluOpType.bypass)
                        nc.vector.tensor_add(out=zsb[:, 0:D_OUT], in0=ztmp[:],
                                             in1=b2sb[:])
                        nc.vector.tensor_copy(
                            out=zkeep[:, t * D_OUT:(t + 1) * D_OUT],
                            in_=zsb[:, 0:D_OUT])
                        nc.sync.dma_start(
                            out=zloc[t * P:(t + 1) * P, :], in_=zsb[:])

            if ph >= 4:
                nc.gpsimd.collective_compute(
                    "AllGather", mybir.AluOpType.bypass, replica_groups=rg,
                    ins=[zloc[:].opt()], outs=[zfull[:].opt()])

            # ---- phase E: decoder in aggregation order
            with (
                tc.tile_pool(name="zsg", bufs=3) as zsgp,
                tc.tile_pool(name="s01", bufs=2) as s01p,
                tc.tile_pool(name="zds", bufs=3) as zdsp,
                tc.tile_pool(name="lac", bufs=1) as lacp,
            ):
                lacc = lacp.tile([P, SB], F32, name="lacc", tag="lacc")
                if ph < 5:
                    nc.gpsimd.memset(lacc[:], 0.0)
                def dec_chunk(g0, zsg, s01, t, r0, o, ch):
                    zdp = php.tile([P, 512], F32, name="zdp", tag="hp")
                    for i in range(ch):
                        rb = r0 + o + i - g0
                        nc.tensor.matmul(
                            zdp[:, i * D_OUT:(i + 1) * D_OUT],
                            lhsT=s01[:, rb * P:(rb + 1) * P],
                            rhs=zkeep[:, t * D_OUT:(t + 1) * D_OUT],
                            start=True, stop=True)
                    zds = zdsp.tile([P, MAXB * Z_PAD], F16, name="zds",
                                    tag="zds")
                    nc.vector.memset(zds[:], 0.0)
                    for i in range(ch):
                        nc.scalar.copy(
                            out=zds[:, i * Z_PAD:i * Z_PAD + D_OUT],
                            in_=zdp[:, i * D_OUT:(i + 1) * D_OUT])
                    prod = zdsp.tile([P, MAXB * Z_PAD], F16, name="prod",
                                     tag="prod")
                    c0 = (r0 + o - g0) * Z_PAD
                    nc.vector.tensor_mul(
                        out=prod[:, 0:ch * Z_PAD],
                        in0=zsg[:, c0:c0 + ch * Z_PAD],
                        in1=zds[:, 0:ch * Z_PAD])
                    nc.vector.reduce_sum(
                        out=lacc[:, r0 + o:r0 + o + ch],
                        in_=prod[:, 0:ch * Z_PAD]
                        .rearrange("p (b e) -> p b e", e=Z_PAD),
                        axis=mybir.AxisListType.X)

                def dec_group(g, tl):
                    g0, nbg, calls = group_info[g]
                    zsg = zsgp.tile([P, nbg_max * Z_PAD], F16, name="zsg",
                                    tag="zsg")
                    issue_gathers(zsg, g0, calls, zfull, Z_PAD)
                    s01 = s01p.tile([P, nbg_max * P], F16, name="s01",
                                    tag="s01")
                    nc.sync.dma_start(
                        out=s01[:, 0:nbg * P],
                        in_=s01t_d[:, g0 * P:(g0 + nbg) * P])
                    for t in tl:
                        for r0, rn in tile_runs(t):
                            o = 0
                            while o < rn:
                                ch = min(MAXB, rn - o)
                                dec_chunk(g0, zsg, s01, t, r0, o, ch)
                                o += ch

                for g, tl in (list(enumerate(groups)) if ph >= 5 else []):
                    dec_group(g, tl)
                nc.sync.dma_start(out=logits_d[:, :], in_=lacc[:])

    nc.compile()
    return nc


# ---------------------------------------------------------------- entry point
_CACHE = {}


def kernel(x, edge_index, W1, b1, W2, b2):
    x = np.asarray(x)
    edge_index = np.asarray(edge_index)
    in_maps, plan, edge_pos = _preprocess(
        x, edge_index, np.asarray(W1), np.asarray(b1),
        np.asarray(W2), np.asarray(b2))
    key = (plan["SB"], tuple(plan["BL"]), tuple(plan["BH"]))
    if key not in _CACHE:
        _CACHE[key] = _build(plan)
    nc = _CACHE[key]
    res = bass_utils.run_bass_kernel_spmd(nc, in_maps, core_ids=list(range(C)))
    out = np.empty(N_EDGES, dtype=np.float32)
    for c in range(C):
        lg = res.results[c]["logits"]           # [P, SB]
        flat = lg.T.reshape(-1)                 # position pb*128+p
        ok = edge_pos[c] >= 0
        out[edge_pos[c][ok]] = flat[ok]
    return out


# revision 16
# speedup vs baseline: 1.1965x; 1.0112x over previous
"""Trainium2 Bass kernel for a 2-layer GCN encoder + edge dot-product decoder.

Math (matches the PyG-style reference):
    deg  = in-degree(dst)+1 (self loops), dinv = rsqrt(deg)
    A~[d,s] = dinv[s]*dinv[d] over edges+self-loops
    H1 = (A~ @ X) @ W1 + b1          (aggregate-first ordering)
    Z  = (A~ @ relu(H1) @ W2) + b2
    logits[e] = <Z[src_e], Z[dst_e]>

Distribution over 8 NeuronCores: nodes sharded via LPT balancing (staged
order), edges partitioned by destination owner, weights replicated.

Key measured constraint: every dynamic row-gather on the SWDGE path costs
~8.9ns/row regardless of batch/row size, serialized on GpSimd.  Design:
  * Layer-1 gather of x[src] rows is STATIC (indices known on host), so the
    host pre-gathers the edge stream into xgath (SBUF layout) and the device
    streams it with static DMA -- zero SWDGE.
  * Layer-2 must gather h2[src] (dynamic, 256B rows) -- batched dma_gather,
    <=1024 rows/call, lo/hi split for signed int16 indices.
  * The decoder runs in the SAME aggregation block structure: z[src] reuses
    the same index table (gathered from zfull), z[dst] is selected from the
    locally-kept z tile via host-shipped one-hot S01T matmuls (zero SWDGE),
    and the dot-products are batched mul + strided 3-D reduce.
The segment-sum scatter is a [128e x 128slot] matrix S (iota is_equal slot
* norm) applied on the Tensor engine with PSUM accumulation.
"""

import os

if os.environ.get("JAX_PLATFORMS") == "cpu":
    os.environ.pop("JAX_PLATFORMS")

import numpy as np

from concourse import bass, bacc, mybir, bass_utils
import concourse.tile as tile

# ---------------------------------------------------------------- sizes
N_NODES = 50000
N_EDGES = 400000
D_IN, D_H, D_OUT = 600, 628, 64
D_PAD = 640                     # x rows padded to 640 f16 = 1280 B
Z_PAD = 128                     # h2/z rows padded to 128 f16 = 256 B
C = 8
P = 128
HALF = 32768                    # int16 signed gather-index window size
B2V = 50176 - HALF              # second window base: [B2V, NALL), idx-B2V
TH = 24                         # tiles in the first AllGather half
MAXB = 8                        # blocks per dma_gather call (8*128=1024 idx)
GT = 4                          # tiles per GEMM group

F16 = mybir.dt.float16
F32 = mybir.dt.float32
I16 = mybir.dt.int16

NPC = N_NODES // C              # 6250
TILES = -(-NPC // P)            # 49
NPAD = TILES * P                # 6272
NALL = C * NPAD                 # 50176


def _chunks(total, step=128):
    out, o = [], 0
    while o < total:
        w = min(step, total - o)
        out.append((o, w))
        o += w
    return out


KCH = _chunks(D_IN)
MCH = _chunks(D_H)


def _wrap16(idx):
    """1-D int array (len % 128 == 0) -> [128, len/16] int16 wrapped layout:
    position i lives at (partition i%16, col i//16), tiled to 128 rows."""
    n = len(idx)
    t = np.asarray(idx, np.int64).reshape(n // 16, 16).T
    t = t.astype(np.uint16).view(np.int16)
    return np.tile(t, (8, 1))


# ---------------------------------------------------------------- host preprocessing
def _assign_nodes(d_all, N, tiles):
    """LPT-assign nodes to C*tiles buckets of <=128 slots, minimizing the
    max per-bucket edge count."""
    import heapq
    w = np.bincount(d_all, minlength=N)
    nb = C * tiles
    heap = [(0, b) for b in range(nb)]
    heapq.heapify(heap)
    cnt = np.zeros(nb, np.int64)
    nodec = np.empty(N, np.int64)
    nodet = np.empty(N, np.int64)
    nodesl = np.empty(N, np.int64)
    for n in np.argsort(-w, kind="stable"):
        while True:
            wt, b = heapq.heappop(heap)
            if cnt[b] < P:
                break
        nodec[n] = b // tiles
        nodet[n] = b % tiles
        nodesl[n] = cnt[b]
        cnt[b] += 1
        if cnt[b] < P:
            heapq.heappush(heap, (wt + int(w[n]), b))
    return nodec, nodet, nodesl


def _preprocess(x, edge_index, W1, b1, W2, b2):
    src = edge_index[0].astype(np.int64)
    dst = edge_index[1].astype(np.int64)
    loop = np.arange(N_NODES, dtype=np.int64)
    s_all = np.concatenate([src, loop])
    d_all = np.concatenate([dst, loop])
    deg = np.bincount(d_all, minlength=N_NODES).astype(np.float64)
    dinv = 1.0 / np.sqrt(deg)
    norm = (dinv[s_all] * dinv[d_all]).astype(np.float32)

    nodec, nodet, nodesl = _assign_nodes(dst, N_NODES, TILES)
    staged = nodec * NPAD + nodet * P + nodesl

    xs = np.zeros((NALL, D_PAD), dtype=np.float16)
    xs[staged, :D_IN] = x.astype(np.float16)

    # per-core staged x rows + self-loop weights (self-loops are folded into
    # the aggregation output instead of occupying block slots)
    inv_staged = np.zeros(NALL, np.int64)
    used = np.zeros(NALL, bool)
    inv_staged[staged] = np.arange(N_NODES)
    used[staged] = True
    dinv2all = np.where(used, 1.0 / deg[inv_staged], 0.0).astype(np.float32)
    dinv2col = dinv2all.reshape(C, TILES, P).transpose(0, 2, 1)  # [C,P,TILES]
    xtl = xs.reshape(C, TILES, P, D_PAD).transpose(0, 2, 1, 3).reshape(
        C, P, TILES * D_PAD)

    # ---- aggregation blocks (REAL edges only): per (core, tile).
    # Two overlapping int16 windows: w1=[0,HALF), w2=[B2V,NALL).  Sources in
    # [B2V,HALF) are flexible; assign just enough of them to w1 to pad each
    # (core,tile) w1 count to a full multiple of 128 (shared block count K1).
    norm = norm[:N_EDGES]
    ec = nodec[dst]
    et = nodet[dst]
    eslot = nodesl[dst]
    esrc = staged[src]
    cls3 = np.where(esrc < B2V, 0, np.where(esrc < HALF, 1, 2))
    key3 = (ec * TILES + et) * 3 + cls3
    cnt3 = np.bincount(key3, minlength=C * TILES * 3).reshape(C, TILES, 3)
    n1, nf, n2 = cnt3[:, :, 0], cnt3[:, :, 1], cnt3[:, :, 2]
    K1 = -(-n1.max(axis=0) // P)                     # [TILES]
    # raise K1 within flex headroom so each GT-group's lo-block sum is a
    # multiple of MAXB (full 1024-row gather calls, fewer call boundaries)
    Kmax = np.minimum((n1 + nf).min(axis=0) // P, -(-(n1 + nf + n2).max(axis=0) // P))
    for i in range(0, TILES, GT):
        tl = list(range(i, min(i + GT, TILES)))
        need = (-int(K1[tl].sum())) % MAXB
        for t in tl:
            room = int(Kmax[t] - K1[t])
            add = min(room, need)
            K1[t] += add
            need -= add
            if need == 0:
                break
    a = np.minimum(np.maximum(K1[None, :] * P - n1, 0), nf)   # flex -> w1
    ehi_cnt1 = n1 + a
    ehi_cnt2 = n2 + nf - a
    BL = np.maximum(-(-ehi_cnt1.max(axis=0) // P), K1)
    BH = -(-ehi_cnt2.max(axis=0) // P)
    # per-edge window: strict by cls3; flex edges -> w1 iff flex-rank < a[c,t]
    ord3 = np.argsort(key3, kind="stable")
    st3 = np.zeros(C * TILES * 3 + 1, np.int64)
    st3[1:] = np.cumsum(cnt3.reshape(-1))
    rank3 = np.arange(N_EDGES) - st3[key3[ord3]]
    ehi_o = np.empty(N_EDGES, np.int64)
    c3o = cls3[ord3]
    ehi_o[c3o == 0] = 0
    ehi_o[c3o == 2] = 1
    fm = c3o == 1
    ehi_o[fm] = (rank3[fm] >= a[ec[ord3][fm], et[ord3][fm]]).astype(np.int64)
    ehi = np.empty(N_EDGES, np.int64)
    ehi[ord3] = ehi_o
    key = (ec * TILES + et) * 2 + ehi
    order = np.argsort(key, kind="stable")
    cnt = np.bincount(key, minlength=C * TILES * 2).reshape(C, TILES, 2)

    groups = [list(range(i, min(i + GT, TILES))) for i in range(0, TILES, GT)]
    bufpos_lo = np.zeros(TILES, np.int64)
    bufpos_hi = np.zeros(TILES, np.int64)
    group_info = []   # per group: (g0pos, nbg, calls=[(cls, bufoff, nb)])
    pos = 0
    for tl in groups:
        g0 = pos
        for t in tl:
            bufpos_lo[t] = pos
            pos += BL[t]
        lo_nb = pos - g0
        for t in tl:
            bufpos_hi[t] = pos
            pos += BH[t]
        hi_nb = pos - g0 - lo_nb
        calls = []
        for cls, coff, cnb in ((0, 0, lo_nb), (1, lo_nb, hi_nb)):
            o = 0
            while o < cnb:
                nb = min(MAXB, cnb - o)
                calls.append((cls, coff + o, nb))
                o += nb
        group_info.append((g0, pos - g0, calls))
    SB = pos

    start = np.zeros(C * TILES * 2 + 1, np.int64)
    start[1:] = np.cumsum(cnt.reshape(-1))
    rank = np.arange(len(order)) - start[key[order]]
    base = np.where(ehi[order] == 0, bufpos_lo[et[order]], bufpos_hi[et[order]])
    col = base + rank // P
    pp = rank % P
    cs = ec[order]
    flat = col * P + pp

    meta = np.zeros((C, P, 2 * SB), dtype=np.float32)
    meta[cs, pp, 2 * col] = eslot[order].astype(np.float32)
    meta[cs, pp, 2 * col + 1] = norm[order]

    aggi = np.zeros((C, SB * P), dtype=np.int64)
    shifted = esrc[order] - ehi[order] * B2V
    edge_pos = np.full((C, SB * P), -1, np.int64)
    s01t = np.zeros((C, P, SB * P), dtype=np.float16)
    for c in range(C):
        m = cs == c
        aggi[c][flat[m]] = shifted[m]
        edge_pos[c][flat[m]] = order[m]
        s01t[c][eslot[order][m], flat[m]] = 1.0
    widx = np.stack([_wrap16(aggi[c]) for c in range(C)])

    # hi-ness per buffer block (to undo the -HALF shift when pre-gathering)
    hi_blocks = np.zeros(SB, np.int64)
    for g0, nbg, calls in group_info:
        for cls, boff, nb in calls:
            if cls == 1:
                hi_blocks[g0 + boff:g0 + boff + nb] = 1

    # host pre-gathered layer-1 stream, already in SBUF layout:
    # xgath[p, b*640:(b+1)*640] = xs row of buffer slot (b, p)
    xgath = []
    for c in range(C):
        rows = aggi[c].reshape(SB, P) + hi_blocks[:, None] * B2V
        g = xs[rows.reshape(-1)].reshape(SB, P, D_PAD)
        xgath.append(np.ascontiguousarray(
            g.transpose(1, 0, 2).reshape(P, SB * D_PAD)))

    iota = np.broadcast_to(np.arange(P, dtype=np.float16), (P, P)).copy()
    ident = np.eye(P, dtype=np.float16)

    shared = {
        "w1": np.ascontiguousarray(W1.astype(np.float16)),
        "w2": np.ascontiguousarray(W2.astype(np.float16)),
        "b1c": np.ascontiguousarray(b1.astype(np.float32).reshape(D_H, 1)),
        "b2r": np.ascontiguousarray(
            np.broadcast_to(b2.astype(np.float32), (P, D_OUT))),
        "iota": iota,
        "ident": ident,
    }
    in_maps = []
    for c in range(C):
        m = dict(shared)
        m["xgath"] = xgath[c]
        m["meta"] = np.ascontiguousarray(meta[c])
        m["widx"] = np.ascontiguousarray(widx[c])
        m["s01t"] = np.ascontiguousarray(s01t[c])
        m["xtl"] = np.ascontiguousarray(xtl[c])
        m["dinv2"] = np.ascontiguousarray(dinv2col[c])
        in_maps.append(m)

    plan = dict(
        SB=SB, groups=groups, group_info=group_info,
        BL=[int(v) for v in BL], BH=[int(v) for v in BH],
        bufpos_lo=bufpos_lo, bufpos_hi=bufpos_hi,
    )
    return in_maps, plan, edge_pos


# ---------------------------------------------------------------- device program
def _build(plan, ph=9):
    SB = plan["SB"]
    groups, group_info = plan["groups"], plan["group_info"]
    BL, BH = plan["BL"], plan["BH"]
    bufpos_lo, bufpos_hi = plan["bufpos_lo"], plan["bufpos_hi"]

    nc = bacc.Bacc("TRN2", target_bir_lowering=False, debug=False,
                   enable_asserts=False, num_devices=C)

    xgath_d = nc.dram_tensor("xgath", [P, SB * D_PAD], F16,
                             kind="ExternalInput")
    w1 = nc.dram_tensor("w1", [D_IN, D_H], F16, kind="ExternalInput")
    w2 = nc.dram_tensor("w2", [D_H, D_OUT], F16, kind="ExternalInput")
    b1c = nc.dram_tensor("b1c", [D_H, 1], F32, kind="ExternalInput")
    b2r = nc.dram_tensor("b2r", [P, D_OUT], F32, kind="ExternalInput")
    iota_d = nc.dram_tensor("iota", [P, P], F16, kind="ExternalInput")
    ident_d = nc.dram_tensor("ident", [P, P], F16, kind="ExternalInput")
    meta_d = nc.dram_tensor("meta", [P, 2 * SB], F32, kind="ExternalInput")
    widx_d = nc.dram_tensor("widx", [P, SB * 8], I16, kind="ExternalInput")
    s01t_d = nc.dram_tensor("s01t", [P, SB * P], F16, kind="ExternalInput")
    xtl_d = nc.dram_tensor("xtl", [P, TILES * D_PAD], F16,
                           kind="ExternalInput")
    dinv2_d = nc.dram_tensor("dinv2", [P, TILES], F32, kind="ExternalInput")
    logits_d = nc.dram_tensor("logits", [P, SB], F32, kind="ExternalOutput")

    rg = [list(range(C))]
    nbg_max = max(gi[1] for gi in group_info)

    with tile.TileContext(nc) as tc:
        with (
            tc.tile_pool(name="const", bufs=1) as constp,
            tc.tile_pool(name="meta", bufs=1) as metap,
            tc.tile_pool(name="sblk", bufs=8) as sp,
            tc.tile_pool(name="xagg", bufs=3) as xaggp,
            tc.tile_pool(name="kxn", bufs=2) as kxnp,
            tc.tile_pool(name="h1r", bufs=2) as h1rp,
            tc.tile_pool(name="h2s", bufs=2) as h2sp,
            tc.tile_pool(name="zz", bufs=4) as zp,
            tc.tile_pool(name="pacc", bufs=2, space="PSUM") as pacc,
            tc.tile_pool(name="ptp", bufs=2, space="PSUM") as ptp,
            tc.tile_pool(name="ph", bufs=2, space="PSUM") as php,
            tc.tile_pool(name="dram", bufs=1, space="DRAM") as dramp,
        ):
            # ---- constants / tables
            w1sb = []
            for k, (k0, kw) in enumerate(KCH):
                t = constp.tile([kw, D_H], F16, name=f"w1sb{k}", tag=f"w1sb{k}")
                nc.sync.dma_start(out=t[:], in_=w1[k0:k0 + kw, :])
                w1sb.append(t)
            w2sb, b1sb = [], []
            for m, (m0, mw) in enumerate(MCH):
                t = constp.tile([mw, D_OUT], F16, name=f"w2sb{m}", tag=f"w2sb{m}")
                nc.sync.dma_start(out=t[:], in_=w2[m0:m0 + mw, :])
                w2sb.append(t)
                bt = constp.tile([mw, 1], F32, name=f"b1sb{m}", tag=f"b1sb{m}")
                nc.sync.dma_start(out=bt[:], in_=b1c[m0:m0 + mw, :])
                b1sb.append(bt)
            b2sb = constp.tile([P, D_OUT], F32, name="b2sb", tag="b2sb")
            nc.sync.dma_start(out=b2sb[:], in_=b2r[:, :])
            iot = constp.tile([P, P], F16, name="iot", tag="iot")
            nc.sync.dma_start(out=iot[:], in_=iota_d[:, :])
            idn = constp.tile([P, P], F16, name="idn", tag="idn")
            nc.sync.dma_start(out=idn[:], in_=ident_d[:, :])
            meta_all = metap.tile([P, 2 * SB], F32, name="meta_all",
                                  tag="meta_all")
            nc.sync.dma_start(out=meta_all[:], in_=meta_d[:, :])
            widx_all = metap.tile([P, SB * 8], I16, name="widx_all",
                                  tag="widx_all")
            nc.sync.dma_start(out=widx_all[:], in_=widx_d[:, :])
            zkeep = metap.tile([P, TILES * D_OUT], F16, name="zkeep",
                               tag="zkeep")
            h2keep = metap.tile([P, TILES * D_OUT], F16, name="h2keep",
                                tag="h2keep")
            dinv2sb = metap.tile([P, TILES], F32, name="dinv2sb",
                                 tag="dinv2sb")
            nc.sync.dma_start(out=dinv2sb[:], in_=dinv2_d[:, :])

            h2loc = dramp.tile([NPAD, Z_PAD], F16, name="h2loc", tag="h2loc")
            h2full = dramp.tile([NALL, Z_PAD], F16, name="h2full",
                                tag="h2full", addr_space="Shared")
            zloc = dramp.tile([NPAD, Z_PAD], F16, name="zloc", tag="zloc")
            zfull = dramp.tile([NALL, Z_PAD], F16, name="zfull",
                               tag="zfull", addr_space="Shared")

            def build_s(o, eng=nc.vector):
                s_t = sp.tile([P, P], F16, name="s_t", tag="s_t")
                eng.tensor_scalar(
                    out=s_t[:], in0=iot[:],
                    scalar1=meta_all[:, 2 * o:2 * o + 1],
                    scalar2=meta_all[:, 2 * o + 1:2 * o + 2],
                    op0=mybir.AluOpType.is_equal,
                    op1=mybir.AluOpType.mult)
                return s_t

            def tile_runs(t):
                """[(first_bufpos, n)] runs of tile t's blocks."""
                runs = []
                if BL[t]:
                    runs.append((int(bufpos_lo[t]), BL[t]))
                if BH[t]:
                    runs.append((int(bufpos_hi[t]), BH[t]))
                return runs

            def tile_blocks(t):
                out = []
                for r0, n in tile_runs(t):
                    out.extend(range(r0, r0 + n))
                return [(pb, i == 0, i == len(out) - 1)
                        for i, pb in enumerate(out)]

            def issue_gathers(dst_tile, g0, calls, table, elem):
                for cls, boff, nb in calls:
                    view = table[B2V:, :] if cls else table[:, :]
                    n = nb * P
                    c0 = (g0 + boff) * 8
                    nc.gpsimd.dma_gather(
                        dst_tile[:, boff * elem:(boff + nb) * elem]
                        .rearrange("p (b e) -> p b e", e=elem),
                        view, widx_all[:, c0:c0 + n // 16], n, n, elem)

            # ---- phase A: L1 aggregate + GEMMs (static pre-gathered stream)
            with tc.tile_pool(name="xg", bufs=2) as xgp, \
                 tc.tile_pool(name="xt", bufs=2) as xtp:
                for g, tl in enumerate(groups):
                    g0, nbg, calls = group_info[g]
                    gw = len(tl) * P
                    xg = xgp.tile([P, nbg_max * D_PAD], F16, name="xg",
                                  tag="xg")
                    nc.sync.dma_start(
                        out=xg[:, 0:nbg * D_PAD],
                        in_=xgath_d[:, g0 * D_PAD:(g0 + nbg) * D_PAD])
                    xtg = xtp.tile([P, GT * D_PAD], F16, name="xtg", tag="xtg")
                    nc.sync.dma_start(
                        out=xtg[:, 0:len(tl) * D_PAD],
                        in_=xtl_d[:, tl[0] * D_PAD:(tl[-1] + 1) * D_PAD])
                    kxn = [kxnp.tile([P, gw], F16, name=f"kxn{k}", tag=f"kxn{k}")
                           for k in range(len(KCH))]
                    for j, t in enumerate(tl):
                        acc = pacc.tile([P, D_IN], F32, name="acc", tag="acc")
                        for pb, first, last in tile_blocks(t):
                            s_t = build_s(pb)
                            rb = pb - g0
                            nc.tensor.matmul(
                                acc[:, 0:512], lhsT=s_t[:],
                                rhs=xg[:, rb * D_PAD:rb * D_PAD + 512],
                                start=first, stop=last)
                            nc.tensor.matmul(
                                acc[:, 512:D_IN], lhsT=s_t[:],
                                rhs=xg[:, rb * D_PAD + 512:rb * D_PAD + D_IN],
                                start=first, stop=last)
                        xaggsb = xaggp.tile([P, D_IN], F16, name="xaggsb",
                                            tag="xaggsb")
                        xin = xtg[:, j * D_PAD:j * D_PAD + D_IN]
                        if tile_blocks(t):
                            nc.vector.scalar_tensor_tensor(
                                out=xaggsb[:], in0=xin,
                                scalar=dinv2sb[:, t:t + 1], in1=acc[:],
                                op0=mybir.AluOpType.mult,
                                op1=mybir.AluOpType.add)
                        else:
                            nc.vector.tensor_scalar(
                                out=xaggsb[:], in0=xin,
                                scalar1=dinv2sb[:, t:t + 1], scalar2=None,
                                op0=mybir.AluOpType.mult,
                                op1=mybir.AluOpType.bypass)
                        for k, (k0, kw) in enumerate(KCH):
                            tp = ptp.tile([P, P], F16, name="tp", tag="tp")
                            nc.tensor.transpose(out=tp[:kw, :],
                                                in_=xaggsb[:, k0:k0 + kw],
                                                identity=idn[:])
                            nc.vector.tensor_copy(
                                out=kxn[k][:kw, j * P:(j + 1) * P],
                                in_=tp[:kw, :])
                    h1r = [h1rp.tile([mw, gw], F16, name=f"h1r{m}",
                                     tag=f"h1r{m}")
                           for m, (m0, mw) in enumerate(MCH)]
                    for m, (m0, mw) in enumerate(MCH):
                        hp = php.tile([P, gw], F32, name="hp", tag="hp")
                        for k, (k0, kw) in enumerate(KCH):
                            nc.tensor.matmul(hp[:mw, :],
                                             lhsT=w1sb[k][:, m0:m0 + mw],
                                             rhs=kxn[k][:kw, :],
                                             start=(k == 0),
                                             stop=(k == len(KCH) - 1))
                        nc.scalar.activation(
                            out=h1r[m][:], in_=hp[:mw, :],
                            func=mybir.ActivationFunctionType.Relu,
                            bias=b1sb[m][:], scale=1.0)
                    h2p = php.tile([P, gw], F32, name="h2p", tag="hp")
                    for m, (m0, mw) in enumerate(MCH):
                        nc.tensor.matmul(h2p[:D_OUT, :], lhsT=w2sb[m][:],
                                         rhs=h1r[m][:],
                                         start=(m == 0),
                                         stop=(m == len(MCH) - 1))
                    h2sb = h2sp.tile([D_OUT, gw], F16, name="h2sb", tag="h2sb")
                    nc.scalar.copy(out=h2sb[:], in_=h2p[:D_OUT, :])
                    for j, t in enumerate(tl):
                        tp2 = ptp.tile([P, P], F16, name="tp2", tag="tp")
                        nc.tensor.transpose(out=tp2[:, :D_OUT],
                                            in_=h2sb[:, j * P:(j + 1) * P],
                                            identity=idn[:D_OUT, :D_OUT])
                        h2row = zp.tile([P, D_OUT], F16, name="h2row",
                                        tag="h2row")
                        nc.vector.tensor_copy(out=h2row[:], in_=tp2[:, :D_OUT])
                        nc.vector.tensor_copy(
                            out=h2keep[:, t * D_OUT:(t + 1) * D_OUT],
                            in_=h2row[:])
                        nc.sync.dma_start(
                            out=h2loc[t * P:(t + 1) * P, 0:D_OUT],
                            in_=h2row[:])

            if ph >= 2:
                nc.gpsimd.collective_compute(
                    "AllGather", mybir.AluOpType.bypass, replica_groups=rg,
                    ins=[h2loc[:].opt()], outs=[h2full[:].opt()])

            # ---- phase C: L2 aggregate
            with tc.tile_pool(name="mg", bufs=3) as mgp:
                for g, tl in (list(enumerate(groups)) if ph >= 3 else []):
                    g0, nbg, calls = group_info[g]
                    mg = mgp.tile([P, nbg_max * Z_PAD], F16, name="mg",
                                  tag="mg")
                    issue_gathers(mg, g0, calls, h2full, Z_PAD)
                    for t in tl:
                        acc2 = ptp.tile([P, D_OUT], F32, name="acc2", tag="tp")
                        for pb, first, last in tile_blocks(t):
                            rb = pb - g0
                            s_t = build_s(pb)
                            nc.tensor.matmul(
                                acc2[:], lhsT=s_t[:],
                                rhs=mg[:, rb * Z_PAD:rb * Z_PAD + D_OUT],
                                start=first, stop=last)
                        zsb = zp.tile([P, Z_PAD], F16, name="zsb", tag="zsb")
                        nc.vector.memset(zsb[:, D_OUT:Z_PAD], 0.0)
                        ztmp = zp.tile([P, D_OUT], F32, name="ztmp",
                                       tag="ztmp")
                        h2in = h2keep[:, t * D_OUT:(t + 1) * D_OUT]
                        if tile_blocks(t):
                            nc.vector.scalar_tensor_tensor(
                                out=ztmp[:], in0=h2in,
                                scalar=dinv2sb[:, t:t + 1], in1=acc2[:],
                                op0=mybir.AluOpType.mult,
                                op1=mybir.AluOpType.add)
                        else:
                            nc.vector.tensor_scalar(
                                out=ztmp[:], in0=h2in,
                                scalar1=dinv2sb[:, t:t + 1], scalar2=None,
                                op0=mybir.AluOpType.mult,
                                op1=mybir.AluOpType.bypass)
                        nc.vector.tensor_add(out=zsb[:, 0:D_OUT], in0=ztmp[:],
                                             in1=b2sb[:])
                        nc.vector.tensor_copy(
                            out=zkeep[:, t * D_OUT:(t + 1) * D_OUT],
                            in_=zsb[:, 0:D_OUT])
                        nc.sync.dma_start(
                            out=zloc[t * P:(t + 1) * P, :], in_=zsb[:])

            if ph >= 4:
                nc.gpsimd.collective_compute(
                    "AllGather", mybir.AluOpType.bypass, replica_groups=rg,
                    ins=[zloc[:].opt()], outs=[zfull[:].opt()])

            # ---- phase E: decoder in aggregation order
            with (
                tc.tile_pool(name="zsg", bufs=3) as zsgp,
                tc.tile_pool(name="s01", bufs=2) as s01p,
                tc.tile_pool(name="zds", bufs=4) as zdsp,
                tc.tile_pool(name="lac", bufs=1) as lacp,
            ):
                lacc = lacp.tile([P, SB], F32, name="lacc", tag="lacc")
                if ph < 5:
                    nc.gpsimd.memset(lacc[:], 0.0)
                def dec_chunk(g0, zsg, s01, t, r0, o, ch):
                    zdp = php.tile([P, 512], F32, name="zdp", tag="hp")
                    for i in range(ch):
                        rb = r0 + o + i - g0
                        nc.tensor.matmul(
                            zdp[:, i * D_OUT:(i + 1) * D_OUT],
                            lhsT=s01[:, rb * P:(rb + 1) * P],
                            rhs=zkeep[:, t * D_OUT:(t + 1) * D_OUT],
                            start=True, stop=True)
                    zds = zdsp.tile([P, MAXB * Z_PAD], F16, name="zds",
                                    tag="zds")
                    nc.vector.memset(zds[:], 0.0)
                    for i in range(ch):
                        nc.scalar.copy(
                            out=zds[:, i * Z_PAD:i * Z_PAD + D_OUT],
                            in_=zdp[:, i * D_OUT:(i + 1) * D_OUT])
                    prod = zdsp.tile([P, MAXB * Z_PAD], F16, name="prod",
                                     tag="prod")
                    c0 = (r0 + o - g0) * Z_PAD
                    nc.vector.tensor_mul(
                        out=prod[:, 0:ch * Z_PAD],
                        in0=zsg[:, c0:c0 + ch * Z_PAD],
                        in1=zds[:, 0:ch * Z_PAD])
                    nc.vector.reduce_sum(
                        out=lacc[:, r0 + o:r0 + o + ch],
                        in_=prod[:, 0:ch * Z_PAD]
                        .rearrange("p (b e) -> p b e", e=Z_PAD),
                        axis=mybir.AxisListType.X)

                def dec_group(g, tl):
                    g0, nbg, calls = group_info[g]
                    zsg = zsgp.tile([P, nbg_max * Z_PAD], F16, name="zsg",
                                    tag="zsg")
                    issue_gathers(zsg, g0, calls, zfull, Z_PAD)
                    s01 = s01p.tile([P, nbg_max * P], F16, name="s01",
                                    tag="s01")
                    nc.sync.dma_start(
                        out=s01[:, 0:nbg * P],
                        in_=s01t_d[:, g0 * P:(g0 + nbg) * P])
                    for t in tl:
                        for r0, rn in tile_runs(t):
                            o = 0
                            while o < rn:
                                ch = min(MAXB, rn - o)
                                dec_chunk(g0, zsg, s01, t, r0, o, ch)
                                o += ch

                for g, tl in (list(enumerate(groups)) if ph >= 5 else []):
                    dec_group(g, tl)
                nc.sync.dma_start(out=logits_d[:, :], in_=lacc[:])

    nc.compile()
    return nc


# ---------------------------------------------------------------- entry point
_CACHE = {}


def kernel(x, edge_index, W1, b1, W2, b2):
    x = np.asarray(x)
    edge_index = np.asarray(edge_index)
    in_maps, plan, edge_pos = _preprocess(
        x, edge_index, np.asarray(W1), np.asarray(b1),
        np.asarray(W2), np.asarray(b2))
    key = (plan["SB"], tuple(plan["BL"]), tuple(plan["BH"]))
    if key not in _CACHE:
        _CACHE[key] = _build(plan)
    nc = _CACHE[key]
    res = bass_utils.run_bass_kernel_spmd(nc, in_maps, core_ids=list(range(C)))
    out = np.empty(N_EDGES, dtype=np.float32)
    for c in range(C):
        lg = res.results[c]["logits"]           # [P, SB]
        flat = lg.T.reshape(-1)                 # position pb*128+p
        ok = edge_pos[c] >= 0
        out[edge_pos[c][ok]] = flat[ok]
    return out


# revision 17
# speedup vs baseline: 1.2046x; 1.0068x over previous
"""Trainium2 Bass kernel for a 2-layer GCN encoder + edge dot-product decoder.

Math (matches the PyG-style reference):
    deg  = in-degree(dst)+1 (self loops), dinv = rsqrt(deg)
    A~[d,s] = dinv[s]*dinv[d] over edges+self-loops
    H1 = (A~ @ X) @ W1 + b1          (aggregate-first ordering)
    Z  = (A~ @ relu(H1) @ W2) + b2
    logits[e] = <Z[src_e], Z[dst_e]>

Distribution over 8 NeuronCores: nodes sharded via LPT balancing (staged
order), edges partitioned by destination owner, weights replicated.

Key measured constraint: every dynamic row-gather on the SWDGE path costs
~8.9ns/row regardless of batch/row size, serialized on GpSimd.  Design:
  * Layer-1 gather of x[src] rows is STATIC (indices known on host), so the
    host pre-gathers the edge stream into xgath (SBUF layout) and the device
    streams it with static DMA -- zero SWDGE.
  * Layer-2 must gather h2[src] (dynamic, 256B rows) -- batched dma_gather,
    <=1024 rows/call, lo/hi split for signed int16 indices.
  * The decoder runs in the SAME aggregation block structure: z[src] reuses
    the same index table (gathered from zfull), z[dst] is selected from the
    locally-kept z tile via host-shipped one-hot S01T matmuls (zero SWDGE),
    and the dot-products are batched mul + strided 3-D reduce.
The segment-sum scatter is a [128e x 128slot] matrix S (iota is_equal slot
* norm) applied on the Tensor engine with PSUM accumulation.
"""

import os

if os.environ.get("JAX_PLATFORMS") == "cpu":
    os.environ.pop("JAX_PLATFORMS")

import numpy as np

from concourse import bass, bacc, mybir, bass_utils
import concourse.tile as tile

# ---------------------------------------------------------------- sizes
N_NODES = 50000
N_EDGES = 400000
D_IN, D_H, D_OUT = 600, 628, 64
D_PAD = 640                     # x rows padded to 640 f16 = 1280 B
Z_PAD = 128                     # h2/z rows padded to 128 f16 = 256 B
C = 8
P = 128
HALF = 32768                    # int16 signed gather-index window size
B2V = 50176 - HALF              # second window base: [B2V, NALL), idx-B2V
TH = 24                         # tiles in the first AllGather half
MAXB = 8                        # blocks per dma_gather call (8*128=1024 idx)
GT = 4                          # tiles per GEMM group

F16 = mybir.dt.float16
F32 = mybir.dt.float32
I16 = mybir.dt.int16

NPC = N_NODES // C              # 6250
TILES = -(-NPC // P)            # 49
NPAD = TILES * P                # 6272
NALL = C * NPAD                 # 50176


def _chunks(total, step=128):
    out, o = [], 0
    while o < total:
        w = min(step, total - o)
        out.append((o, w))
        o += w
    return out


KCH = _chunks(D_IN)
MCH = _chunks(D_H)


def _wrap16(idx):
    """1-D int array (len % 128 == 0) -> [128, len/16] int16 wrapped layout:
    position i lives at (partition i%16, col i//16), tiled to 128 rows."""
    n = len(idx)
    t = np.asarray(idx, np.int64).reshape(n // 16, 16).T
    t = t.astype(np.uint16).view(np.int16)
    return np.tile(t, (8, 1))


# ---------------------------------------------------------------- host preprocessing
def _assign_nodes(d_all, N, tiles):
    """LPT-assign nodes to C*tiles buckets of <=128 slots, minimizing the
    max per-bucket edge count."""
    import heapq
    w = np.bincount(d_all, minlength=N)
    nb = C * tiles
    heap = [(0, b) for b in range(nb)]
    heapq.heapify(heap)
    cnt = np.zeros(nb, np.int64)
    nodec = np.empty(N, np.int64)
    nodet = np.empty(N, np.int64)
    nodesl = np.empty(N, np.int64)
    for n in np.argsort(-w, kind="stable"):
        while True:
            wt, b = heapq.heappop(heap)
            if cnt[b] < P:
                break
        nodec[n] = b // tiles
        nodet[n] = b % tiles
        nodesl[n] = cnt[b]
        cnt[b] += 1
        if cnt[b] < P:
            heapq.heappush(heap, (wt + int(w[n]), b))
    return nodec, nodet, nodesl


def _preprocess(x, edge_index, W1, b1, W2, b2):
    src = edge_index[0].astype(np.int64)
    dst = edge_index[1].astype(np.int64)
    loop = np.arange(N_NODES, dtype=np.int64)
    s_all = np.concatenate([src, loop])
    d_all = np.concatenate([dst, loop])
    deg = np.bincount(d_all, minlength=N_NODES).astype(np.float64)
    dinv = 1.0 / np.sqrt(deg)
    norm = (dinv[s_all] * dinv[d_all]).astype(np.float32)

    nodec, nodet, nodesl = _assign_nodes(dst, N_NODES, TILES)
    staged = nodec * NPAD + nodet * P + nodesl

    xs = np.zeros((NALL, D_PAD), dtype=np.float16)
    xs[staged, :D_IN] = x.astype(np.float16)

    # per-core staged x rows + self-loop weights (self-loops are folded into
    # the aggregation output instead of occupying block slots)
    inv_staged = np.zeros(NALL, np.int64)
    used = np.zeros(NALL, bool)
    inv_staged[staged] = np.arange(N_NODES)
    used[staged] = True
    dinv2all = np.where(used, 1.0 / deg[inv_staged], 0.0).astype(np.float32)
    dinv2col = dinv2all.reshape(C, TILES, P)                  # [C,TILES,P]
    sdiag = np.zeros((C, P, TILES * P), dtype=np.float16)
    pp_ = np.arange(P)
    for c in range(C):
        for t in range(TILES):
            sdiag[c][pp_, t * P + pp_] = dinv2col[c, t]
    xtl = xs.reshape(C, TILES, P, D_PAD).transpose(0, 2, 1, 3).reshape(
        C, P, TILES * D_PAD)

    # ---- aggregation blocks (REAL edges only): per (core, tile).
    # Two overlapping int16 windows: w1=[0,HALF), w2=[B2V,NALL).  Sources in
    # [B2V,HALF) are flexible; assign just enough of them to w1 to pad each
    # (core,tile) w1 count to a full multiple of 128 (shared block count K1).
    norm = norm[:N_EDGES]
    ec = nodec[dst]
    et = nodet[dst]
    eslot = nodesl[dst]
    esrc = staged[src]
    cls3 = np.where(esrc < B2V, 0, np.where(esrc < HALF, 1, 2))
    key3 = (ec * TILES + et) * 3 + cls3
    cnt3 = np.bincount(key3, minlength=C * TILES * 3).reshape(C, TILES, 3)
    n1, nf, n2 = cnt3[:, :, 0], cnt3[:, :, 1], cnt3[:, :, 2]
    K1 = -(-n1.max(axis=0) // P)                     # [TILES]
    # raise K1 within flex headroom so each GT-group's lo-block sum is a
    # multiple of MAXB (full 1024-row gather calls, fewer call boundaries)
    Kmax = np.minimum((n1 + nf).min(axis=0) // P, -(-(n1 + nf + n2).max(axis=0) // P))
    for i in range(0, TILES, GT):
        tl = list(range(i, min(i + GT, TILES)))
        need = (-int(K1[tl].sum())) % MAXB
        for t in tl:
            room = int(Kmax[t] - K1[t])
            add = min(room, need)
            K1[t] += add
            need -= add
            if need == 0:
                break
    a = np.minimum(np.maximum(K1[None, :] * P - n1, 0), nf)   # flex -> w1
    ehi_cnt1 = n1 + a
    ehi_cnt2 = n2 + nf - a
    BL = np.maximum(-(-ehi_cnt1.max(axis=0) // P), K1)
    BH = -(-ehi_cnt2.max(axis=0) // P)
    # per-edge window: strict by cls3; flex edges -> w1 iff flex-rank < a[c,t]
    ord3 = np.argsort(key3, kind="stable")
    st3 = np.zeros(C * TILES * 3 + 1, np.int64)
    st3[1:] = np.cumsum(cnt3.reshape(-1))
    rank3 = np.arange(N_EDGES) - st3[key3[ord3]]
    ehi_o = np.empty(N_EDGES, np.int64)
    c3o = cls3[ord3]
    ehi_o[c3o == 0] = 0
    ehi_o[c3o == 2] = 1
    fm = c3o == 1
    ehi_o[fm] = (rank3[fm] >= a[ec[ord3][fm], et[ord3][fm]]).astype(np.int64)
    ehi = np.empty(N_EDGES, np.int64)
    ehi[ord3] = ehi_o
    key = (ec * TILES + et) * 2 + ehi
    order = np.argsort(key, kind="stable")
    cnt = np.bincount(key, minlength=C * TILES * 2).reshape(C, TILES, 2)

    groups = [list(range(i, min(i + GT, TILES))) for i in range(0, TILES, GT)]
    bufpos_lo = np.zeros(TILES, np.int64)
    bufpos_hi = np.zeros(TILES, np.int64)
    group_info = []   # per group: (g0pos, nbg, calls=[(cls, bufoff, nb)])
    pos = 0
    for tl in groups:
        g0 = pos
        for t in tl:
            bufpos_lo[t] = pos
            pos += BL[t]
        lo_nb = pos - g0
        for t in tl:
            bufpos_hi[t] = pos
            pos += BH[t]
        hi_nb = pos - g0 - lo_nb
        calls = []
        for cls, coff, cnb in ((0, 0, lo_nb), (1, lo_nb, hi_nb)):
            o = 0
            while o < cnb:
                nb = min(MAXB, cnb - o)
                calls.append((cls, coff + o, nb))
                o += nb
        group_info.append((g0, pos - g0, calls))
    SB = pos

    start = np.zeros(C * TILES * 2 + 1, np.int64)
    start[1:] = np.cumsum(cnt.reshape(-1))
    rank = np.arange(len(order)) - start[key[order]]
    base = np.where(ehi[order] == 0, bufpos_lo[et[order]], bufpos_hi[et[order]])
    col = base + rank // P
    pp = rank % P
    cs = ec[order]
    flat = col * P + pp

    meta = np.zeros((C, P, 2 * SB), dtype=np.float32)
    meta[cs, pp, 2 * col] = eslot[order].astype(np.float32)
    meta[cs, pp, 2 * col + 1] = norm[order]

    aggi = np.zeros((C, SB * P), dtype=np.int64)
    shifted = esrc[order] - ehi[order] * B2V
    edge_pos = np.full((C, SB * P), -1, np.int64)
    s01t = np.zeros((C, P, SB * P), dtype=np.float16)
    for c in range(C):
        m = cs == c
        aggi[c][flat[m]] = shifted[m]
        edge_pos[c][flat[m]] = order[m]
        s01t[c][eslot[order][m], flat[m]] = 1.0
    widx = np.stack([_wrap16(aggi[c]) for c in range(C)])

    # hi-ness per buffer block (to undo the -HALF shift when pre-gathering)
    hi_blocks = np.zeros(SB, np.int64)
    for g0, nbg, calls in group_info:
        for cls, boff, nb in calls:
            if cls == 1:
                hi_blocks[g0 + boff:g0 + boff + nb] = 1

    # host pre-gathered layer-1 stream, already in SBUF layout:
    # xgath[p, b*640:(b+1)*640] = xs row of buffer slot (b, p)
    xgath = []
    for c in range(C):
        rows = aggi[c].reshape(SB, P) + hi_blocks[:, None] * B2V
        g = xs[rows.reshape(-1)].reshape(SB, P, D_PAD)
        xgath.append(np.ascontiguousarray(
            g.transpose(1, 0, 2).reshape(P, SB * D_PAD)))

    iota = np.broadcast_to(np.arange(P, dtype=np.float16), (P, P)).copy()
    ident = np.eye(P, dtype=np.float16)

    shared = {
        "w1": np.ascontiguousarray(W1.astype(np.float16)),
        "w2": np.ascontiguousarray(W2.astype(np.float16)),
        "b1c": np.ascontiguousarray(b1.astype(np.float32).reshape(D_H, 1)),
        "b2r": np.ascontiguousarray(
            np.broadcast_to(b2.astype(np.float32), (P, D_OUT))),
        "iota": iota,
        "ident": ident,
    }
    in_maps = []
    for c in range(C):
        m = dict(shared)
        m["xgath"] = xgath[c]
        m["meta"] = np.ascontiguousarray(meta[c])
        m["widx"] = np.ascontiguousarray(widx[c])
        m["s01t"] = np.ascontiguousarray(s01t[c])
        m["xtl"] = np.ascontiguousarray(xtl[c])
        m["sdiag"] = np.ascontiguousarray(sdiag[c])
        in_maps.append(m)

    plan = dict(
        SB=SB, groups=groups, group_info=group_info,
        BL=[int(v) for v in BL], BH=[int(v) for v in BH],
        bufpos_lo=bufpos_lo, bufpos_hi=bufpos_hi,
    )
    return in_maps, plan, edge_pos


# ---------------------------------------------------------------- device program
def _build(plan, ph=9):
    SB = plan["SB"]
    groups, group_info = plan["groups"], plan["group_info"]
    BL, BH = plan["BL"], plan["BH"]
    bufpos_lo, bufpos_hi = plan["bufpos_lo"], plan["bufpos_hi"]

    nc = bacc.Bacc("TRN2", target_bir_lowering=False, debug=False,
                   enable_asserts=False, num_devices=C)

    xgath_d = nc.dram_tensor("xgath", [P, SB * D_PAD], F16,
                             kind="ExternalInput")
    w1 = nc.dram_tensor("w1", [D_IN, D_H], F16, kind="ExternalInput")
    w2 = nc.dram_tensor("w2", [D_H, D_OUT], F16, kind="ExternalInput")
    b1c = nc.dram_tensor("b1c", [D_H, 1], F32, kind="ExternalInput")
    b2r = nc.dram_tensor("b2r", [P, D_OUT], F32, kind="ExternalInput")
    iota_d = nc.dram_tensor("iota", [P, P], F16, kind="ExternalInput")
    ident_d = nc.dram_tensor("ident", [P, P], F16, kind="ExternalInput")
    meta_d = nc.dram_tensor("meta", [P, 2 * SB], F32, kind="ExternalInput")
    widx_d = nc.dram_tensor("widx", [P, SB * 8], I16, kind="ExternalInput")
    s01t_d = nc.dram_tensor("s01t", [P, SB * P], F16, kind="ExternalInput")
    xtl_d = nc.dram_tensor("xtl", [P, TILES * D_PAD], F16,
                           kind="ExternalInput")
    sdiag_d = nc.dram_tensor("sdiag", [P, TILES * P], F16,
                             kind="ExternalInput")
    logits_d = nc.dram_tensor("logits", [P, SB], F32, kind="ExternalOutput")

    rg = [list(range(C))]
    nbg_max = max(gi[1] for gi in group_info)

    with tile.TileContext(nc) as tc:
        with (
            tc.tile_pool(name="const", bufs=1) as constp,
            tc.tile_pool(name="meta", bufs=1) as metap,
            tc.tile_pool(name="sblk", bufs=8) as sp,
            tc.tile_pool(name="xagg", bufs=3) as xaggp,
            tc.tile_pool(name="kxn", bufs=2) as kxnp,
            tc.tile_pool(name="h1r", bufs=2) as h1rp,
            tc.tile_pool(name="h2s", bufs=2) as h2sp,
            tc.tile_pool(name="zz", bufs=4) as zp,
            tc.tile_pool(name="pacc", bufs=2, space="PSUM") as pacc,
            tc.tile_pool(name="ptp", bufs=2, space="PSUM") as ptp,
            tc.tile_pool(name="ph", bufs=2, space="PSUM") as php,
            tc.tile_pool(name="dram", bufs=1, space="DRAM") as dramp,
        ):
            # ---- constants / tables
            w1sb = []
            for k, (k0, kw) in enumerate(KCH):
                t = constp.tile([kw, D_H], F16, name=f"w1sb{k}", tag=f"w1sb{k}")
                nc.sync.dma_start(out=t[:], in_=w1[k0:k0 + kw, :])
                w1sb.append(t)
            w2sb, b1sb = [], []
            for m, (m0, mw) in enumerate(MCH):
                t = constp.tile([mw, D_OUT], F16, name=f"w2sb{m}", tag=f"w2sb{m}")
                nc.sync.dma_start(out=t[:], in_=w2[m0:m0 + mw, :])
                w2sb.append(t)
                bt = constp.tile([mw, 1], F32, name=f"b1sb{m}", tag=f"b1sb{m}")
                nc.sync.dma_start(out=bt[:], in_=b1c[m0:m0 + mw, :])
                b1sb.append(bt)
            b2sb = constp.tile([P, D_OUT], F32, name="b2sb", tag="b2sb")
            nc.sync.dma_start(out=b2sb[:], in_=b2r[:, :])
            iot = constp.tile([P, P], F16, name="iot", tag="iot")
            nc.sync.dma_start(out=iot[:], in_=iota_d[:, :])
            idn = constp.tile([P, P], F16, name="idn", tag="idn")
            nc.sync.dma_start(out=idn[:], in_=ident_d[:, :])
            meta_all = metap.tile([P, 2 * SB], F32, name="meta_all",
                                  tag="meta_all")
            nc.sync.dma_start(out=meta_all[:], in_=meta_d[:, :])
            widx_all = metap.tile([P, SB * 8], I16, name="widx_all",
                                  tag="widx_all")
            nc.sync.dma_start(out=widx_all[:], in_=widx_d[:, :])
            zkeep = metap.tile([P, TILES * D_OUT], F16, name="zkeep",
                               tag="zkeep")
            h2keep = metap.tile([P, TILES * D_OUT], F16, name="h2keep",
                                tag="h2keep")
            sdg = metap.tile([P, TILES * P], F16, name="sdg", tag="sdg")
            nc.sync.dma_start(out=sdg[:], in_=sdiag_d[:, :])

            h2loc = dramp.tile([NPAD, Z_PAD], F16, name="h2loc", tag="h2loc")
            h2full = dramp.tile([NALL, Z_PAD], F16, name="h2full",
                                tag="h2full", addr_space="Shared")
            zloc = dramp.tile([NPAD, Z_PAD], F16, name="zloc", tag="zloc")
            zfull = dramp.tile([NALL, Z_PAD], F16, name="zfull",
                               tag="zfull", addr_space="Shared")

            def build_s(o, eng=nc.vector):
                s_t = sp.tile([P, P], F16, name="s_t", tag="s_t")
                eng.tensor_scalar(
                    out=s_t[:], in0=iot[:],
                    scalar1=meta_all[:, 2 * o:2 * o + 1],
                    scalar2=meta_all[:, 2 * o + 1:2 * o + 2],
                    op0=mybir.AluOpType.is_equal,
                    op1=mybir.AluOpType.mult)
                return s_t

            def tile_runs(t):
                """[(first_bufpos, n)] runs of tile t's blocks."""
                runs = []
                if BL[t]:
                    runs.append((int(bufpos_lo[t]), BL[t]))
                if BH[t]:
                    runs.append((int(bufpos_hi[t]), BH[t]))
                return runs

            def tile_blocks(t):
                out = []
                for r0, n in tile_runs(t):
                    out.extend(range(r0, r0 + n))
                return [(pb, i == 0, i == len(out) - 1)
                        for i, pb in enumerate(out)]

            def issue_gathers(dst_tile, g0, calls, table, elem):
                for cls, boff, nb in calls:
                    view = table[B2V:, :] if cls else table[:, :]
                    n = nb * P
                    c0 = (g0 + boff) * 8
                    nc.gpsimd.dma_gather(
                        dst_tile[:, boff * elem:(boff + nb) * elem]
                        .rearrange("p (b e) -> p b e", e=elem),
                        view, widx_all[:, c0:c0 + n // 16], n, n, elem)

            # ---- phase A: L1 aggregate + GEMMs (static pre-gathered stream)
            with tc.tile_pool(name="xg", bufs=2) as xgp, \
                 tc.tile_pool(name="xt", bufs=2) as xtp:
                for g, tl in enumerate(groups):
                    g0, nbg, calls = group_info[g]
                    gw = len(tl) * P
                    xg = xgp.tile([P, nbg_max * D_PAD], F16, name="xg",
                                  tag="xg")
                    nc.sync.dma_start(
                        out=xg[:, 0:nbg * D_PAD],
                        in_=xgath_d[:, g0 * D_PAD:(g0 + nbg) * D_PAD])
                    xtg = xtp.tile([P, GT * D_PAD], F16, name="xtg", tag="xtg")
                    nc.sync.dma_start(
                        out=xtg[:, 0:len(tl) * D_PAD],
                        in_=xtl_d[:, tl[0] * D_PAD:(tl[-1] + 1) * D_PAD])
                    kxn = [kxnp.tile([P, gw], F16, name=f"kxn{k}", tag=f"kxn{k}")
                           for k in range(len(KCH))]
                    for j, t in enumerate(tl):
                        acc = pacc.tile([P, D_IN], F32, name="acc", tag="acc")
                        blocks = tile_blocks(t)
                        dlhs = sdg[:, t * P:(t + 1) * P]
                        xin = xtg[:, j * D_PAD:j * D_PAD + D_IN]
                        nc.tensor.matmul(acc[:, 0:512], lhsT=dlhs,
                                         rhs=xin[:, 0:512],
                                         start=True, stop=not blocks)
                        nc.tensor.matmul(acc[:, 512:D_IN], lhsT=dlhs,
                                         rhs=xin[:, 512:D_IN],
                                         start=True, stop=not blocks)
                        for pb, first, last in blocks:
                            s_t = build_s(pb)
                            rb = pb - g0
                            nc.tensor.matmul(
                                acc[:, 0:512], lhsT=s_t[:],
                                rhs=xg[:, rb * D_PAD:rb * D_PAD + 512],
                                start=False, stop=last)
                            nc.tensor.matmul(
                                acc[:, 512:D_IN], lhsT=s_t[:],
                                rhs=xg[:, rb * D_PAD + 512:rb * D_PAD + D_IN],
                                start=False, stop=last)
                        xaggsb = xaggp.tile([P, D_IN], F16, name="xaggsb",
                                            tag="xaggsb")
                        nc.scalar.copy(out=xaggsb[:], in_=acc[:])
                        for k, (k0, kw) in enumerate(KCH):
                            tp = ptp.tile([P, P], F16, name="tp", tag="tp")
                            nc.tensor.transpose(out=tp[:kw, :],
                                                in_=xaggsb[:, k0:k0 + kw],
                                                identity=idn[:])
                            nc.vector.tensor_copy(
                                out=kxn[k][:kw, j * P:(j + 1) * P],
                                in_=tp[:kw, :])
                    h1r = [h1rp.tile([mw, gw], F16, name=f"h1r{m}",
                                     tag=f"h1r{m}")
                           for m, (m0, mw) in enumerate(MCH)]
                    for m, (m0, mw) in enumerate(MCH):
                        hp = php.tile([P, gw], F32, name="hp", tag="hp")
                        for k, (k0, kw) in enumerate(KCH):
                            nc.tensor.matmul(hp[:mw, :],
                                             lhsT=w1sb[k][:, m0:m0 + mw],
                                             rhs=kxn[k][:kw, :],
                                             start=(k == 0),
                                             stop=(k == len(KCH) - 1))
                        nc.scalar.activation(
                            out=h1r[m][:], in_=hp[:mw, :],
                            func=mybir.ActivationFunctionType.Relu,
                            bias=b1sb[m][:], scale=1.0)
                    h2p = php.tile([P, gw], F32, name="h2p", tag="hp")
                    for m, (m0, mw) in enumerate(MCH):
                        nc.tensor.matmul(h2p[:D_OUT, :], lhsT=w2sb[m][:],
                                         rhs=h1r[m][:],
                                         start=(m == 0),
                                         stop=(m == len(MCH) - 1))
                    h2sb = h2sp.tile([D_OUT, gw], F16, name="h2sb", tag="h2sb")
                    nc.scalar.copy(out=h2sb[:], in_=h2p[:D_OUT, :])
                    for j, t in enumerate(tl):
                        tp2 = ptp.tile([P, P], F16, name="tp2", tag="tp")
                        nc.tensor.transpose(out=tp2[:, :D_OUT],
                                            in_=h2sb[:, j * P:(j + 1) * P],
                                            identity=idn[:D_OUT, :D_OUT])
                        h2row = zp.tile([P, D_OUT], F16, name="h2row",
                                        tag="h2row")
                        nc.vector.tensor_copy(out=h2row[:], in_=tp2[:, :D_OUT])
                        nc.vector.tensor_copy(
                            out=h2keep[:, t * D_OUT:(t + 1) * D_OUT],
                            in_=h2row[:])
                        nc.sync.dma_start(
                            out=h2loc[t * P:(t + 1) * P, 0:D_OUT],
                            in_=h2row[:])

            if ph >= 2:
                nc.gpsimd.collective_compute(
                    "AllGather", mybir.AluOpType.bypass, replica_groups=rg,
                    ins=[h2loc[:].opt()], outs=[h2full[:].opt()])

            # ---- phase C: L2 aggregate
            with tc.tile_pool(name="mg", bufs=3) as mgp:
                for g, tl in (list(enumerate(groups)) if ph >= 3 else []):
                    g0, nbg, calls = group_info[g]
                    mg = mgp.tile([P, nbg_max * Z_PAD], F16, name="mg",
                                  tag="mg")
                    issue_gathers(mg, g0, calls, h2full, Z_PAD)
                    for t in tl:
                        acc2 = ptp.tile([P, D_OUT], F32, name="acc2", tag="tp")
                        blocks = tile_blocks(t)
                        nc.tensor.matmul(
                            acc2[:], lhsT=sdg[:, t * P:(t + 1) * P],
                            rhs=h2keep[:, t * D_OUT:(t + 1) * D_OUT],
                            start=True, stop=not blocks)
                        for pb, first, last in blocks:
                            rb = pb - g0
                            s_t = build_s(pb)
                            nc.tensor.matmul(
                                acc2[:], lhsT=s_t[:],
                                rhs=mg[:, rb * Z_PAD:rb * Z_PAD + D_OUT],
                                start=False, stop=last)
                        zsb = zp.tile([P, Z_PAD], F16, name="zsb", tag="zsb")
                        nc.vector.memset(zsb[:, D_OUT:Z_PAD], 0.0)
                        nc.vector.tensor_add(out=zsb[:, 0:D_OUT], in0=acc2[:],
                                             in1=b2sb[:])
                        nc.vector.tensor_copy(
                            out=zkeep[:, t * D_OUT:(t + 1) * D_OUT],
                            in_=zsb[:, 0:D_OUT])
                        nc.sync.dma_start(
                            out=zloc[t * P:(t + 1) * P, :], in_=zsb[:])

            if ph >= 4:
                nc.gpsimd.collective_compute(
                    "AllGather", mybir.AluOpType.bypass, replica_groups=rg,
                    ins=[zloc[:].opt()], outs=[zfull[:].opt()])

            # ---- phase E: decoder in aggregation order
            with (
                tc.tile_pool(name="zsg", bufs=3) as zsgp,
                tc.tile_pool(name="s01", bufs=2) as s01p,
                tc.tile_pool(name="zds", bufs=4) as zdsp,
                tc.tile_pool(name="lac", bufs=1) as lacp,
            ):
                lacc = lacp.tile([P, SB], F32, name="lacc", tag="lacc")
                if ph < 5:
                    nc.gpsimd.memset(lacc[:], 0.0)
                def dec_chunk(g0, zsg, s01, t, r0, o, ch):
                    zdp = php.tile([P, 512], F32, name="zdp", tag="hp")
                    for i in range(ch):
                        rb = r0 + o + i - g0
                        nc.tensor.matmul(
                            zdp[:, i * D_OUT:(i + 1) * D_OUT],
                            lhsT=s01[:, rb * P:(rb + 1) * P],
                            rhs=zkeep[:, t * D_OUT:(t + 1) * D_OUT],
                            start=True, stop=True)
                    zds = zdsp.tile([P, MAXB * Z_PAD], F16, name="zds",
                                    tag="zds")
                    nc.vector.memset(zds[:], 0.0)
                    for i in range(ch):
                        nc.scalar.copy(
                            out=zds[:, i * Z_PAD:i * Z_PAD + D_OUT],
                            in_=zdp[:, i * D_OUT:(i + 1) * D_OUT])
                    prod = zdsp.tile([P, MAXB * Z_PAD], F16, name="prod",
                                     tag="prod")
                    c0 = (r0 + o - g0) * Z_PAD
                    nc.vector.tensor_mul(
                        out=prod[:, 0:ch * Z_PAD],
                        in0=zsg[:, c0:c0 + ch * Z_PAD],
                        in1=zds[:, 0:ch * Z_PAD])
                    nc.vector.reduce_sum(
                        out=lacc[:, r0 + o:r0 + o + ch],
                        in_=prod[:, 0:ch * Z_PAD]
                        .rearrange("p (b e) -> p b e", e=Z_PAD),
                        axis=mybir.AxisListType.X)

                def dec_group(g, tl):
                    g0, nbg, calls = group_info[g]
                    zsg = zsgp.tile([P, nbg_max * Z_PAD], F16, name="zsg",
                                    tag="zsg")
                    issue_gathers(zsg, g0, calls, zfull, Z_PAD)
                    s01 = s01p.tile([P, nbg_max * P], F16, name="s01",
                                    tag="s01")
                    nc.sync.dma_start(
                        out=s01[:, 0:nbg * P],
                        in_=s01t_d[:, g0 * P:(g0 + nbg) * P])
                    for t in tl:
                        for r0, rn in tile_runs(t):
                            o = 0
                            while o < rn:
                                ch = min(MAXB, rn - o)
                                dec_chunk(g0, zsg, s01, t, r0, o, ch)
                                o += ch

                for g, tl in (list(enumerate(groups)) if ph >= 5 else []):
                    dec_group(g, tl)
                nc.sync.dma_start(out=logits_d[:, :], in_=lacc[:])

    nc.compile()
    return nc


# ---------------------------------------------------------------- entry point
_CACHE = {}


def kernel(x, edge_index, W1, b1, W2, b2):
    x = np.asarray(x)
    edge_index = np.asarray(edge_index)
    in_maps, plan, edge_pos = _preprocess(
        x, edge_index, np.asarray(W1), np.asarray(b1),
        np.asarray(W2), np.asarray(b2))
    key = (plan["SB"], tuple(plan["BL"]), tuple(plan["BH"]))
    if key not in _CACHE:
        _CACHE[key] = _build(plan)
    nc = _CACHE[key]
    res = bass_utils.run_bass_kernel_spmd(nc, in_maps, core_ids=list(range(C)))
    out = np.empty(N_EDGES, dtype=np.float32)
    for c in range(C):
        lg = res.results[c]["logits"]           # [P, SB]
        flat = lg.T.reshape(-1)                 # position pb*128+p
        ok = edge_pos[c] >= 0
        out[edge_pos[c][ok]] = flat[ok]
    return out
